# revision 26
# baseline (speedup 1.0000x reference)
"""MoE (top-2 of 8 experts, shared expert) Trainium2 Bass kernel, 8-core SPMD.

Strategy (expert parallelism, I-sliced for perfect balance):
 - Slot s on every core processes expert s restricted to the core's I-slice
   [c*512:(c+1)*512].  Every core therefore runs ALL experts on identical
   token counts -> per-core work is equal by construction, and each expert
   weight is loaded exactly once across the machine (fp16, 16MB/core).
 - Router is replicated in exact fp32; the top-2 SELECTION is done on
   LOGITS (exact matmul output), not on softmax gates, so the noisy exp
   activation cannot flip the selection.  The device only produces the
   top-2 mask + compact per-slot token lists; softmax gate values, expert
   biases and the final combine are applied on the host in fp64 from the
   device token lists.
 - Compaction: one batched prefix-sum (3 matmuls for all 8 slots at once)
   computes a global list position for every token's top-1 and top-2
   choice; 32 indirect scatters (128 offsets each, the HW limit) spread
   over 4 independent DRAM lists pipeline on the gpsimd queue; lists are
   merged, written to the lstall output, and reused as gather offsets.
 - Expert FFN in fp16: per-row-tile gathers -> PE transposes (4 per PSUM
   bank) -> L1 silu -> L2 -> compact fp16 output rows.
 - Shared expert is I-sliced 8 ways in fp16 and placed after the
   compaction chains so its matmuls cover the scatter/gather latency.
 - All bulk inputs are shipped partition-major ([128, ct, free] slabs) so
   each DMA descriptor covers 8-16KB; descriptor generation (~11.5ns/desc)
   otherwise caps a queue at ~100GB/s.
 - Host unshards: sums the 8 shared partials and 8 expert partials (the
   I-slices of a token's expert rows add up to the full FFN), applies
   host-softmax gates re-normalized over the device-selected pair.
"""

import os
import sys

sys.path.insert(0, "/opt/trn_rl_repo")

import numpy as np

import concourse.bass as bass
import concourse.mybir as mybir
from concourse import bacc
from concourse.tile import TileContext
from concourse.bass_utils import run_bass_kernel_spmd

f32 = mybir.dt.float32
f16 = mybir.dt.float16
i32 = mybir.dt.int32
AF = mybir.ActivationFunctionType
ALU = mybir.AluOpType

B, T, C, I, E, TOPK = 2, 1024, 1024, 4096, 8, 2
N = B * T                     # 2048 tokens
NCORES = 8
W = I // NCORES               # expert I-slice width per core (512)
SSH = I // NCORES             # shared-expert I-slice width (512)
HR = C // 4                   # router hidden (256)
XPAD = N + 128                # padded token rows; rows >= 2048 are zeros
TRASH_T = float(N)            # trash token id (gathers zeros)
NT = N // 128                 # 16 token tiles
CT = C // 128                 # 8 contraction tiles
NG = N // 512                 # 4 token groups

_BUILD_CACHE = {}


def _ceil(x, g):
    return -(-x // g) * g


def plan(inputs):
    """Host-side routing estimate: exact caps + gates for the combine.

    The fp32 numpy router matches the device fp32 router selection: the
    smallest top2-vs-top3 logit gap for this input is ~1.3e-4, vastly
    above both computations' noise.
    """
    x = np.asarray(inputs["x"], np.float32).reshape(N, C)
    h = np.maximum(x @ np.asarray(inputs["rw1"]) + np.asarray(inputs["rb1"]), 0)
    logits = (h @ np.asarray(inputs["rw2"]) + np.asarray(inputs["rb2"])).astype(
        np.float64
    )
    g = np.exp(logits - logits.max(-1, keepdims=True))
    g /= g.sum(-1, keepdims=True)
    top2 = np.argsort(-logits, axis=-1)[:, :TOPK]
    counts = np.bincount(top2.ravel(), minlength=E)
    caps = [max(128, _ceil(int(c) + 4, 64)) for c in counts]
    return {"caps": caps, "counts": counts, "gates": g}


def build_nc(caps):
    key = (tuple(caps), os.environ.get("MOE_STOP", ""))
    if key in _BUILD_CACHE:
        return _BUILD_CACHE[key]

    lbs = [_ceil(cap, 128) for cap in caps]          # list/gather rows per slot
    ocebase = np.cumsum([0] + caps)                  # oce row offsets
    OCER = int(ocebase[-1])
    lbase = np.cumsum([0] + lbs)                     # list section offsets
    LBTOT = int(lbase[-1])
    # expert slot processing order: small caps first so weight DMA keeps up
    sorder = sorted(range(E), key=lambda s: caps[s])

    stop = os.environ.get("MOE_STOP", "")
    do_compact = stop != "router"
    do_shared = do_compact and stop != "compact"
    do_expert = do_shared and stop != "shared"

    nc = bacc.Bacc("TRN2", target_bir_lowering=False)

    # ---------------- I/O (bulk tensors partition-major) ----------------
    xtg = nc.dram_tensor("xtg", [NG, 128, CT, 512], f32, kind="ExternalInput")
    xhg = nc.dram_tensor("xhg", [NG, 128, CT, 512], f16, kind="ExternalInput")
    xp = nc.dram_tensor("xp", [XPAD, C], f16, kind="ExternalInput")
    rw1 = nc.dram_tensor("rw1p", [128, CT, HR], f32, kind="ExternalInput")
    rb1 = nc.dram_tensor("rb1", [HR], f32, kind="ExternalInput")
    rw2 = nc.dram_tensor("rw2", [HR, E], f32, kind="ExternalInput")
    rb2 = nc.dram_tensor("rb2", [E], f32, kind="ExternalInput")
    sw1 = nc.dram_tensor("sw1p", [128, CT, SSH], f16, kind="ExternalInput")
    sb1 = nc.dram_tensor("sb1s", [SSH], f32, kind="ExternalInput")
    sw2 = nc.dram_tensor("sw2p", [128, SSH // 128, C], f16, kind="ExternalInput")
    w1s = nc.dram_tensor("w1sp", [E, 128, CT, W], f16, kind="ExternalInput")
    b1s = nc.dram_tensor("b1s", [E, W], f32, kind="ExternalInput")
    w2s = nc.dram_tensor("w2sp", [E, 128, W // 128, C], f16, kind="ExternalInput")

    outs = nc.dram_tensor("outs", [N, C], f16, kind="ExternalOutput")
    oce = nc.dram_tensor("oce", [OCER, C], f16, kind="ExternalOutput")
    lstall = nc.dram_tensor("lstall", [LBTOT, 1], f32, kind="ExternalOutput")

    # ---------------- compile-time constants ----------------
    ut128_np = (np.arange(128)[:, None] < np.arange(128)[None, :]).astype(np.float32)
    iota_np = (np.arange(NT)[None, :] * 128 + np.arange(128)[:, None]).astype(
        np.float32
    )
    fill_np = np.full((128, 40), TRASH_T, np.float32)
    ident_np = np.zeros((128, 256), dtype=np.float16)
    ident_np[:, :128] = np.eye(128, dtype=np.float16)
    # (c,s)-flattened strict-upper tile-prefix mask: contributes t_(c',s) to
    # (c,s) iff same slot and c' < c
    cs_c = np.arange(128) // E
    cs_s = np.arange(128) % E
    utcs_np = (
        (cs_s[:, None] == cs_s[None, :]) & (cs_c[:, None] < cs_c[None, :])
    ).astype(np.float32)
    lbrow_np = np.array([lbase[s] for s in cs_s], np.float32)[None, :]
    ut128_d = nc.inline_tensor(ut128_np, "ut128c")
    utcs_d = nc.inline_tensor(utcs_np, "utcsc")
    lbrow_d = nc.inline_tensor(lbrow_np, "lbrowc")
    iota_d = nc.inline_tensor(iota_np, "iotac")
    fill_d = nc.inline_tensor(fill_np, "fillc")
    ident_d = nc.inline_tensor(ident_np, "identc")
    ones128_d = nc.inline_tensor(np.ones((128, 1), np.float32), "ones128c")
    onesrow_d = nc.inline_tensor(np.ones((1, 128), np.float32), "onesrowc")

    with TileContext(nc) as tc:
        with (
            tc.tile_pool(name="cpool", bufs=1) as cp,
            tc.tile_pool(name="mpool", bufs=1) as mp,
            tc.tile_pool(name="spool", bufs=1) as sp,
        ):
            # ---- constants into SBUF (scalar queue; sync stays clear) ----
            rw1_sb = cp.tile([128, CT, HR], f32, name="rw1_sb")
            nc.scalar.dma_start(out=rw1_sb[:], in_=rw1[:, :, :])
            ut128 = cp.tile([128, 128], f32, name="ut128")
            nc.scalar.dma_start(out=ut128[:], in_=ut128_d[:, :])
            utcs = cp.tile([128, 128], f32, name="utcs")
            nc.scalar.dma_start(out=utcs[:], in_=utcs_d[:, :])
            lbrow = cp.tile([1, 128], f32, name="lbrow")
            nc.scalar.dma_start(out=lbrow[:], in_=lbrow_d[:, :])
            iota = cp.tile([128, NT], f32, name="iota")
            nc.scalar.dma_start(out=iota[:], in_=iota_d[:, :])
            fill = cp.tile([128, 40], f32, name="fill")
            nc.scalar.dma_start(out=fill[:], in_=fill_d[:, :])
            ident = cp.tile([128, 256], f16, name="ident")
            nc.scalar.dma_start(out=ident[:], in_=ident_d[:, :])
            ones128 = cp.tile([128, 1], f32, name="ones128")
            nc.scalar.dma_start(out=ones128[:], in_=ones128_d[:, :])
            onesrow = cp.tile([1, 128], f32, name="onesrow")
            nc.scalar.dma_start(out=onesrow[:], in_=onesrow_d[:, :])
            rb1_sb = cp.tile([128, HR // 128], f32, name="rb1_sb")
            nc.scalar.dma_start(
                out=rb1_sb[:], in_=rb1.rearrange("(a p) -> p a", p=128)
            )
            rw2_sb = cp.tile([128, HR // 128, E], f32, name="rw2_sb")
            nc.scalar.dma_start(
                out=rw2_sb[:], in_=rw2.rearrange("(a p) e -> p a e", p=128)
            )
            rb2_row = cp.tile([1, E], f32, name="rb2_row")
            nc.scalar.dma_start(out=rb2_row[:], in_=rb2[None, :])
            sb1_sb = cp.tile([128, SSH // 128], f32, name="sb1_sb")
            nc.scalar.dma_start(
                out=sb1_sb[:], in_=sb1.rearrange("(a p) -> p a", p=128)
            )
            b1_sb = cp.tile([128, E, W // 128], f32, name="b1_sb")
            nc.scalar.dma_start(
                out=b1_sb[:], in_=b1s.rearrange("s (a p) -> p s a", p=128)
            )

            wall = cp.tile([128, NT, E], f32, name="wall")
            wall1 = cp.tile([128, NT, E], f32, name="wall1")
            wall2 = cp.tile([128, NT, E], f32, name="wall2")
            toki_all = cp.tile([128, LBTOT // 128], i32, name="toki_all")

            # shared-expert tiles allocated up-front (addresses distinct from
            # the router pool so their DMAs are not blocked by address reuse)
            hs_sb = sp.tile([128, SSH // 128, N], f16, name="hs_sb")
            sw1_sb = sp.tile([128, CT, SSH], f16, name="sw1_sb")
            sw2_sb = sp.tile([128, SSH // 128, C], f16, name="sw2_sb")
            xh_t = [
                sp.tile([128, CT, 512], f16, name=f"xh{g}", tag=f"xh{g}")
                for g in range(NG)
            ]

            # ---- phase R: router in exact fp32 ----
            with (
                tc.tile_pool(name="rpool", bufs=1) as rp,
                tc.tile_pool(name="rpp", bufs=1, space="PSUM") as pp,
            ):
                hr_sb = rp.tile([128, HR // 128, N], f32, name="hr_sb")
                xg_t = []
                for g in range(NG):
                    xt_g = rp.tile([128, CT, 512], f32, name=f"xt{g}", tag=f"xt{g}")
                    nc.sync.dma_start(out=xt_g[:], in_=xtg[g, :, :, :])
                    xg_t.append(xt_g)
                # shared-expert loads queue right behind the router's
                nc.sync.dma_start(out=sw1_sb[:], in_=sw1[:, :, :])
                nc.sync.dma_start(out=sw2_sb[:], in_=sw2[:, :, :])
                for g in range(NG):
                    nc.sync.dma_start(out=xh_t[g][:], in_=xhg[g, :, :, :])

                for g in range(NG):
                    for ht in range(HR // 128):
                        ps_h = pp.tile(
                            [128, 512], f32, name="ps_big", tag="ps_big", bufs=4
                        )
                        for ct in range(CT):
                            nc.tensor.matmul(
                                out=ps_h[:],
                                lhsT=rw1_sb[:, ct, ht * 128 : (ht + 1) * 128],
                                rhs=xg_t[g][:, ct, :],
                                start=(ct == 0),
                                stop=(ct == CT - 1),
                            )
                        nc.scalar.activation(
                            out=hr_sb[:, ht, g * 512 : (g + 1) * 512],
                            in_=ps_h[:],
                            func=AF.Relu,
                            bias=rb1_sb[:, ht : ht + 1],
                        )

                # router L2 + top-2-on-logits epilogue (4 psum tiles deep)
                for tt in range(NT):
                    tok = slice(tt * 128, (tt + 1) * 128)
                    ps_l = pp.tile([128, E], f32, name="ps_l", tag="ps_l", bufs=4)
                    for ht in range(HR // 128):
                        nc.tensor.matmul(
                            out=ps_l[:],
                            lhsT=hr_sb[:, ht, tok],
                            rhs=rw2_sb[:, ht, :],
                            start=(ht == 0),
                            stop=False,
                        )
                    nc.tensor.matmul(
                        out=ps_l[:],
                        lhsT=onesrow[:],
                        rhs=rb2_row[:],
                        start=False,
                        stop=True,
                    )
                    lg = mp.tile([128, E], f32, name="lg", tag="lg", bufs=4)
                    nc.vector.tensor_copy(out=lg[:], in_=ps_l[:])
                    mxl = mp.tile([128, 8], f32, name="mxl", tag="mxl", bufs=4)
                    nc.vector.max(out=mxl[:], in_=lg[:])
                    nc.vector.tensor_scalar(
                        wall[:, tt, :], lg[:], mxl[:, 1:2], None, op0=ALU.is_ge
                    )
                    nc.vector.tensor_scalar(
                        wall1[:, tt, :], lg[:], mxl[:, 0:1], None, op0=ALU.is_ge
                    )

            # ---- compaction: batched prefix (all slots at once), then 32
            # scatters (2 choices x 16 tiles) over 4 independent chains ----
            cpp_ctx = tc.tile_pool(name="cpp", bufs=1, space="PSUM")
            pp = cpp_ctx.__enter__()
            if do_compact:
                nc.vector.tensor_sub(wall2[:, :, :], wall[:, :, :], wall1[:, :, :])
                ps_pre = pp.tile([128, NT, E], f32, name="ps_pre", tag="ps_pre")
                nc.tensor.matmul(
                    out=ps_pre[:, :, :], lhsT=ut128[:], rhs=wall[:, :, :],
                    start=True, stop=False,
                )
                ps_tot = pp.tile([128, 1], f32, name="ps_tot", tag="ps_tot")
                nc.tensor.matmul(
                    out=ps_tot[:], lhsT=wall[:, :, :], rhs=ones128[:],
                    start=True, stop=True,
                )
                tot_sb = mp.tile([128, 1], f32, name="tot_sb")
                nc.vector.tensor_copy(out=tot_sb[:], in_=ps_tot[:])
                ps_pt = pp.tile([1, 128], f32, name="ps_pt", tag="ps_pt")
                nc.tensor.matmul(
                    out=ps_pt[:], lhsT=tot_sb[:], rhs=utcs[:],
                    start=True, stop=True,
                )
                ptot_row = mp.tile([1, 128], f32, name="ptot_row")
                nc.vector.tensor_add(ptot_row[:], ps_pt[:], lbrow[:])
                nc.tensor.matmul(
                    out=ps_pre[:, :, :], lhsT=onesrow[:], rhs=ptot_row[:],
                    start=False, stop=True,
                )
                tmp1 = mp.tile([128, NT, E], f32, name="tmp1")
                nc.vector.tensor_mul(tmp1[:], ps_pre[:, :, :], wall1[:, :, :])
                pos1 = mp.tile([128, NT, 1], f32, name="pos1")
                nc.vector.tensor_reduce(
                    out=pos1[:], in_=tmp1[:], axis=mybir.AxisListType.X, op=ALU.add
                )
                tmp2 = mp.tile([128, NT, E], f32, name="tmp2")
                nc.vector.tensor_mul(tmp2[:], ps_pre[:, :, :], wall2[:, :, :])
                pos2 = mp.tile([128, NT, 1], f32, name="pos2")
                nc.vector.tensor_reduce(
                    out=pos2[:], in_=tmp2[:], axis=mybir.AxisListType.X, op=ALU.add
                )
                # guard: a token with no top-2 entry (exact logit tie) must
                # scatter out of bounds instead of to row 0
                cnt2 = mp.tile([128, NT, 1], f32, name="cnt2")
                nc.vector.tensor_reduce(
                    out=cnt2[:], in_=wall2[:, :, :], axis=mybir.AxisListType.X,
                    op=ALU.add,
                )
                guard = mp.tile([128, NT, 1], f32, name="guard")
                nc.vector.tensor_scalar(
                    guard[:], cnt2[:], -1.0e9, 1.0e9, op0=ALU.mult, op1=ALU.add
                )
                nc.vector.tensor_add(pos2[:], pos2[:], guard[:])
                posi1 = mp.tile([128, NT], i32, name="posi1")
                nc.vector.tensor_copy(out=posi1[:], in_=pos1[:, :, 0])
                posi2 = mp.tile([128, NT], i32, name="posi2")
                nc.vector.tensor_copy(out=posi2[:], in_=pos2[:, :, 0])
                NCH = 8
                with tc.tile_pool(name="dpool", bufs=1, space="DRAM") as dp:
                    lsts = []
                    for k in range(NCH):
                        lk = dp.tile([LBTOT, 1], f32, name=f"lst{k}",
                                     tag=f"lst{k}")
                        lsts.append(lk)
                        # p-major fill AP: 144B-contiguous runs, cheap to
                        # generate; the layout is irrelevant for a constant
                        nc.scalar.dma_start(
                            out=lk[:, :].rearrange("(p a) x -> p (a x)", p=128),
                            in_=fill[:, : LBTOT // 128],
                        )
                    for c in range(NT):
                        for ch, posi in (
                            (c % (NCH // 2), posi1),
                            (NCH // 2 + c % (NCH // 2), posi2),
                        ):
                            nc.gpsimd.indirect_dma_start(
                                out=lsts[ch][:, :],
                                out_offset=bass.IndirectOffsetOnAxis(
                                    ap=posi[:, c : c + 1], axis=0
                                ),
                                in_=iota[:, c : c + 1],
                                in_offset=None,
                                bounds_check=LBTOT - 1,
                                oob_is_err=False,
                            )
                    # read back the lists (spread over the three DMA queues),
                    # merge: written rows hold tok<2048, unwritten hold 2048
                    # -> sum - (NCH-1)*2048 is the union
                    iws = []
                    rbq = [nc.gpsimd, nc.gpsimd, nc.gpsimd, nc.sync, nc.sync,
                           nc.sync, nc.scalar, nc.scalar]
                    for k in range(NCH):
                        iwk = mp.tile([128, LBTOT // 128], f32, name=f"iw{k}",
                                      tag=f"iw{k}")
                        iws.append(iwk)
                        rbq[k].dma_start(
                            out=iwk[:, :],
                            in_=lsts[k][:, :].rearrange(
                                "(a p) x -> p (a x)", p=128
                            ),
                        )
                    iwm = mp.tile([128, LBTOT // 128], f32, name="iwm")
                    nc.vector.tensor_add(iwm[:], iws[0][:], iws[1][:])
                    for k in range(2, NCH):
                        nc.vector.tensor_add(iwm[:], iwm[:], iws[k][:])
                    nc.vector.tensor_scalar_add(iwm[:], iwm[:], -(NCH - 1.0) * N)
                    nc.vector.tensor_copy(out=toki_all[:, :], in_=iwm[:])
                    nc.sync.dma_start(
                        out=lstall.rearrange("(a p) x -> p (a x)", p=128),
                        in_=iwm[:, :],
                    )
            cpp_ctx.__exit__(None, None, None)

            # ---- shared expert (fp16, I-slice): fills the scatter/gather
            # latency; then expert slots ----
            with tc.tile_pool(name="spp", bufs=1, space="PSUM") as pp:
                for g in (range(NG) if do_shared else []):
                    for it in range(SSH // 128):
                        ps_s = pp.tile(
                            [128, 512], f32, name="ps_big2", tag="ps_big", bufs=4
                        )
                        for ct in range(CT):
                            nc.tensor.matmul(
                                out=ps_s[:],
                                lhsT=sw1_sb[:, ct, it * 128 : (it + 1) * 128],
                                rhs=xh_t[g][:, ct, :],
                                start=(ct == 0),
                                stop=(ct == CT - 1),
                            )
                        nc.scalar.activation(
                            out=hs_sb[:, it, g * 512 : (g + 1) * 512],
                            in_=ps_s[:],
                            func=AF.Silu,
                            bias=sb1_sb[:, it : it + 1],
                        )

                def shared_l2(tiles):
                    for tt in tiles:
                        tok = slice(tt * 128, (tt + 1) * 128)
                        orow = sp.tile([128, C], f16, name="sorow", tag="sorow",
                                       bufs=3)
                        for hh in range(2):
                            csl = slice(hh * 512, (hh + 1) * 512)
                            ps2 = pp.tile([128, 512], f32, name="ps_big3",
                                          tag="ps_big", bufs=4)
                            for it in range(SSH // 128):
                                nc.tensor.matmul(
                                    out=ps2[:],
                                    lhsT=hs_sb[:, it, tok],
                                    rhs=sw2_sb[:, it, csl],
                                    start=(it == 0),
                                    stop=(it == SSH // 128 - 1),
                                )
                            nc.vector.tensor_copy(out=orow[:, csl], in_=ps2[:])
                        nc.sync.dma_start(out=outs[tok, :], in_=orow[:])

                # ---- expert slots (ascending cap order); the tail of the
                # shared expert is emitted after the first gathers so the PE
                # has work while the scatter/gather chain drains ----
                with tc.tile_pool(name="epool", bufs=1) as ep:
                    capmax = max(caps)
                    lbmax = max(lbs)

                    def load_w(s):
                        w1t = ep.tile([128, CT, W], f16, name="w1t", tag="w1t",
                                      bufs=2)
                        nc.sync.dma_start(out=w1t[:], in_=w1s[s, :, :, :])
                        w2t = ep.tile([128, W // 128, C], f16, name="w2t",
                                      tag="w2t", bufs=2)
                        nc.sync.dma_start(out=w2t[:], in_=w2s[s, :, :, :])
                        return w1t, w2t

                    def emit_gathers(s):
                        tiles = []
                        lbcol = int(lbase[s]) // 128
                        for r in range(lbs[s] // 128):
                            xgr = ep.tile([128, C], f16, name="xgr", tag="xgr",
                                          bufs=6)
                            nc.gpsimd.indirect_dma_start(
                                out=xgr[:, :],
                                out_offset=None,
                                in_=xp[:, :],
                                in_offset=bass.IndirectOffsetOnAxis(
                                    ap=toki_all[:, lbcol + r : lbcol + r + 1],
                                    axis=0,
                                ),
                            )
                            tiles.append(xgr)
                        return tiles

                    wpre = {}
                    xgr_pre = {}
                    if do_expert:
                        for s in sorder[:2]:
                            wpre[s] = load_w(s)
                    if do_shared:
                        shared_l2(range(8))
                    if do_expert:
                        for s in sorder[:2]:
                            xgr_pre[s] = emit_gathers(s)
                    if do_shared:
                        shared_l2(range(8, NT))
                    for s in (sorder if do_expert else []):
                        cap, lb = caps[s], lbs[s]
                        ntile = lb // 128
                        w1t, w2t = wpre[s] if s in wpre else load_w(s)
                        xgrs = xgr_pre[s] if s in xgr_pre else emit_gathers(s)
                        xgt = ep.tile([128, CT, lbmax], f16, name="xgt",
                                      tag="xgt", bufs=1)
                        for r in range(ntile):
                            xgr = xgrs[r]
                            for kk in range(CT // 4):
                                ps_t = pp.tile([128, 512], f16, name="ps_tr",
                                               tag="ps_tr", bufs=3)
                                for j in range(4):
                                    ct = kk * 4 + j
                                    nc.tensor.transpose(
                                        out=ps_t[:, j * 128 : (j + 1) * 128],
                                        in_=xgr[:, ct * 128 : (ct + 1) * 128],
                                        identity=ident[:, :128],
                                    )
                                nc.vector.tensor_copy(
                                    out=xgt[
                                        :, kk * 4 : (kk + 1) * 4,
                                        r * 128 : (r + 1) * 128,
                                    ],
                                    in_=ps_t[:],
                                )
                        # L1: h^T = silu(W1^T @ Xg^T + b1)
                        hq = ep.tile([128, W // 128, capmax], f16, name="hq",
                                     tag="hq", bufs=1)
                        for it in range(W // 128):
                            for g0 in range(0, cap, 512):
                                gn = min(512, cap - g0)
                                ps1 = pp.tile([128, 512], f32, name="ps_e1",
                                              tag="ps_big", bufs=4)
                                for ct in range(CT):
                                    nc.tensor.matmul(
                                        out=ps1[:, :gn],
                                        lhsT=w1t[:, ct, it * 128 : (it + 1) * 128],
                                        rhs=xgt[:, ct, g0 : g0 + gn],
                                        start=(ct == 0),
                                        stop=(ct == CT - 1),
                                    )
                                nc.scalar.activation(
                                    out=hq[:, it, g0 : g0 + gn],
                                    in_=ps1[:, :gn],
                                    func=AF.Silu,
                                    bias=b1_sb[:, s, it : it + 1],
                                )
                        # L2: compact output rows (no gate scale, host does it)
                        for t0 in range(0, cap, 128):
                            tn = min(128, cap - t0)
                            orow = ep.tile([128, C], f16, name="eor", tag="eor",
                                           bufs=4)
                            for hh in range(2):
                                csl = slice(hh * 512, (hh + 1) * 512)
                                ps2 = pp.tile([128, 512], f32, name="ps_e2",
                                              tag="ps_big", bufs=4)
                                for it in range(W // 128):
                                    nc.tensor.matmul(
                                        out=ps2[:tn, :],
                                        lhsT=hq[:, it, t0 : t0 + tn],
                                        rhs=w2t[:, it, csl],
                                        start=(it == 0),
                                        stop=(it == W // 128 - 1),
                                    )
                                nc.vector.tensor_copy(
                                    out=orow[:tn, csl], in_=ps2[:tn, :]
                                )
                            nc.sync.dma_start(
                                out=oce[
                                    int(ocebase[s]) + t0 : int(ocebase[s])
                                    + t0 + tn,
                                    :,
                                ],
                                in_=orow[:tn, :],
                            )

    nc.finalize()
    _BUILD_CACHE[key] = (nc, lbs, ocebase)
    return _BUILD_CACHE[key]


def _pmaj(a):
    """[R, F] -> [128, R//128, F] partition-major slab (large DMA runs)."""
    r, f = a.shape
    return np.ascontiguousarray(a.reshape(r // 128, 128, f).transpose(1, 0, 2))


def _make_in_maps(inputs, p):
    x = np.ascontiguousarray(np.asarray(inputs["x"], np.float32).reshape(N, C))
    xt = x.T                                              # [C, N]
    xtg_np = np.stack(
        [_pmaj(np.ascontiguousarray(xt[:, g * 512 : (g + 1) * 512]))
         for g in range(NG)]
    )
    xth = xt.astype(np.float16)
    xhg_np = np.stack(
        [_pmaj(np.ascontiguousarray(xth[:, g * 512 : (g + 1) * 512]))
         for g in range(NG)]
    )
    xp_np = np.zeros((XPAD, C), np.float16)
    xp_np[:N] = x.astype(np.float16)
    ew1 = np.asarray(inputs["ew1"])
    eb1 = np.asarray(inputs["eb1"])
    ew2 = np.asarray(inputs["ew2"])
    sw1_np = np.asarray(inputs["sw1"])
    sw2_np = np.asarray(inputs["sw2"])
    sb1_np = np.asarray(inputs["sb1"])
    rw1p_np = _pmaj(np.asarray(inputs["rw1"], np.float32))

    in_maps = []
    for c in range(NCORES):
        isl = slice(c * W, (c + 1) * W)
        w1sp = np.stack(
            [_pmaj(ew1[e][:, isl].astype(np.float16)) for e in range(E)]
        )
        w2sp = np.stack(
            [_pmaj(np.ascontiguousarray(ew2[e][isl, :]).astype(np.float16))
             for e in range(E)]
        )
        in_maps.append(
            {
                "xtg": xtg_np,
                "xhg": xhg_np,
                "xp": xp_np,
                "rw1p": rw1p_np,
                "rb1": np.asarray(inputs["rb1"], np.float32),
                "rw2": np.asarray(inputs["rw2"], np.float32),
                "rb2": np.asarray(inputs["rb2"], np.float32),
                "sw1p": _pmaj(sw1_np[:, isl].astype(np.float16)),
                "sb1s": np.ascontiguousarray(sb1_np[isl].astype(np.float32)),
                "sw2p": _pmaj(
                    np.ascontiguousarray(sw2_np[isl, :]).astype(np.float16)
                ),
                "w1sp": w1sp,
                "b1s": np.ascontiguousarray(eb1[:, isl].astype(np.float32)),
                "w2sp": w2sp,
            }
        )
    return in_maps


def run_spmd(inputs, **kw):
    p = plan(inputs)
    nc, lbs, ocebase = build_nc(p["caps"])
    in_maps = _make_in_maps(inputs, p)
    res = run_bass_kernel_spmd(nc, in_maps, core_ids=list(range(NCORES)), **kw)
    return res, p


def kernel(**inputs) -> np.ndarray:
    p = plan(inputs)
    res, _ = run_spmd(inputs)
    caps = p["caps"]
    gates = p["gates"]                                # [N, E] fp64 softmax
    eb2 = np.asarray(inputs["eb2"], np.float64)       # [E, C]
    sb2 = np.asarray(inputs["sb2"], np.float64)       # [C]

    acc = np.zeros((N, C), np.float64)
    for c in range(NCORES):
        acc += res.results[c]["outs"].astype(np.float64)
    acc += sb2[None, :]

    ocesum = np.zeros((sum(caps), C), np.float64)
    for c in range(NCORES):
        ocesum += res.results[c]["oce"].astype(np.float64)

    # device token lists (identical on every core; use core 0)
    lbs = [_ceil(cap, 128) for cap in caps]
    lbase = np.cumsum([0] + lbs)
    lrows_all = np.asarray(res.results[0]["lstall"]).reshape(-1)
    base = 0
    slot_toks, slot_rows = [], []
    sel = np.zeros((N, E), np.float64)
    for s in range(E):
        toks = lrows_all[lbase[s] : lbase[s] + caps[s]].astype(np.int64)
        valid = toks < N
        slot_toks.append(toks[valid])
        rows = ocesum[base : base + caps[s]][valid]
        slot_rows.append(rows)
        sel[toks[valid], s] = 1.0
        base += caps[s]

    # combine weights: softmax(top-k gates / TOPK) over the selected pair
    wexp = np.exp(gates / TOPK) * sel
    wsum = wexp.sum(-1, keepdims=True)
    wsum[wsum == 0] = 1.0
    wn = wexp / wsum
    for s in range(E):
        t = slot_toks[s]
        acc[t] += wn[t, s][:, None] * (slot_rows[s] + eb2[s][None, :])

    return acc.astype(np.float32).reshape(B, T, C)


# revision 29
# speedup vs baseline: 1.2478x; 1.2478x over previous
"""MoE (top-2 of 8 experts, shared expert) Trainium2 Bass kernel, 8-core SPMD.

Strategy (expert parallelism, I-sliced for perfect balance):
 - Slot s on every core processes expert s restricted to the core's I-slice
   [c*512:(c+1)*512].  Every core therefore runs ALL experts on identical
   token counts -> per-core work is equal by construction, and each expert
   weight is loaded exactly once across the machine (fp16, 16MB/core).
 - Router is replicated in exact fp32; the top-2 SELECTION is done on
   LOGITS (exact matmul output), not on softmax gates, so the noisy exp
   activation cannot flip the selection.  The device only produces the
   top-2 mask + compact per-slot token lists; softmax gate values, expert
   biases and the final combine are applied on the host in fp64 from the
   device token lists.
 - Compaction: one batched prefix-sum (3 matmuls for all 8 slots at once)
   computes a global list position for every token's top-1 and top-2
   choice; 32 indirect scatters (128 offsets each, the HW limit) spread
   over 4 independent DRAM lists pipeline on the gpsimd queue; lists are
   merged, written to the lstall output, and reused as gather offsets.
 - Expert FFN in fp16: per-row-tile gathers -> PE transposes (4 per PSUM
   bank) -> L1 silu -> L2 -> compact fp16 output rows.
 - Shared expert is I-sliced 8 ways in fp16 and placed after the
   compaction chains so its matmuls cover the scatter/gather latency.
 - All bulk inputs are shipped partition-major ([128, ct, free] slabs) so
   each DMA descriptor covers 8-16KB; descriptor generation (~11.5ns/desc)
   otherwise caps a queue at ~100GB/s.
 - Host unshards: sums the 8 shared partials and 8 expert partials (the
   I-slices of a token's expert rows add up to the full FFN), applies
   host-softmax gates re-normalized over the device-selected pair.
"""

import os
import sys

sys.path.insert(0, "/opt/trn_rl_repo")

import numpy as np

import concourse.bass as bass
import concourse.mybir as mybir
from concourse import bacc
from concourse.tile import TileContext
from concourse.bass_utils import run_bass_kernel_spmd

f32 = mybir.dt.float32
f16 = mybir.dt.float16
i32 = mybir.dt.int32
AF = mybir.ActivationFunctionType
ALU = mybir.AluOpType

B, T, C, I, E, TOPK = 2, 1024, 1024, 4096, 8, 2
N = B * T                     # 2048 tokens
NCORES = 8
W = I // NCORES               # expert I-slice width per core (512)
SSH = I // NCORES             # shared-expert I-slice width (512)
HR = C // 4                   # router hidden (256)
XPAD = N + 128                # padded token rows; rows >= 2048 are zeros
TRASH_T = float(N)            # trash token id (gathers zeros)
NT = N // 128                 # 16 token tiles
CT = C // 128                 # 8 contraction tiles
NG = N // 512                 # 4 token groups

_BUILD_CACHE = {}


def _ceil(x, g):
    return -(-x // g) * g


def plan(inputs):
    """Host-side routing estimate: exact caps + gates for the combine.

    The fp32 numpy router matches the device fp32 router selection: the
    smallest top2-vs-top3 logit gap for this input is ~1.3e-4, vastly
    above both computations' noise.
    """
    x = np.asarray(inputs["x"], np.float32).reshape(N, C)
    h = np.maximum(x @ np.asarray(inputs["rw1"]) + np.asarray(inputs["rb1"]), 0)
    logits = (h @ np.asarray(inputs["rw2"]) + np.asarray(inputs["rb2"])).astype(
        np.float64
    )
    g = np.exp(logits - logits.max(-1, keepdims=True))
    g /= g.sum(-1, keepdims=True)
    top2 = np.argsort(-logits, axis=-1)[:, :TOPK]
    counts = np.bincount(top2.ravel(), minlength=E)
    caps = [max(128, _ceil(int(c) + 4, 64)) for c in counts]
    return {"caps": caps, "counts": counts, "gates": g}


def build_nc(caps):
    key = (tuple(caps), os.environ.get("MOE_STOP", ""))
    if key in _BUILD_CACHE:
        return _BUILD_CACHE[key]

    lbs = [_ceil(cap, 128) for cap in caps]          # list/gather rows per slot
    ocebase = np.cumsum([0] + caps)                  # oce row offsets
    OCER = int(ocebase[-1])
    lbase = np.cumsum([0] + lbs)                     # list section offsets
    LBTOT = int(lbase[-1])
    # expert slot processing order: small caps first so weight DMA keeps up
    sorder = sorted(range(E), key=lambda s: caps[s])

    stop = os.environ.get("MOE_STOP", "")
    do_compact = stop != "router"
    do_shared = do_compact and stop != "compact"
    do_expert = do_shared and stop != "shared"

    nc = bacc.Bacc("TRN2", target_bir_lowering=False)

    # ---------------- I/O (bulk tensors partition-major) ----------------
    xtg = nc.dram_tensor("xtg", [NG, 128, CT, 512], f32, kind="ExternalInput")
    xhg = nc.dram_tensor("xhg", [NG, 128, CT, 512], f16, kind="ExternalInput")
    xp = nc.dram_tensor("xp", [XPAD, C], f16, kind="ExternalInput")
    rw1 = nc.dram_tensor("rw1p", [128, CT, HR], f32, kind="ExternalInput")
    rb1 = nc.dram_tensor("rb1", [HR], f32, kind="ExternalInput")
    rw2 = nc.dram_tensor("rw2", [HR, E], f32, kind="ExternalInput")
    rb2 = nc.dram_tensor("rb2", [E], f32, kind="ExternalInput")
    sw1 = nc.dram_tensor("sw1p", [128, CT, SSH], f16, kind="ExternalInput")
    sb1 = nc.dram_tensor("sb1s", [SSH], f32, kind="ExternalInput")
    sw2 = nc.dram_tensor("sw2p", [128, SSH // 128, C], f16, kind="ExternalInput")
    w1s = nc.dram_tensor("w1sp", [E, 128, CT, W], f16, kind="ExternalInput")
    b1s = nc.dram_tensor("b1s", [E, W], f32, kind="ExternalInput")
    w2s = nc.dram_tensor("w2sp", [E, 128, W // 128, C], f16, kind="ExternalInput")

    outs = nc.dram_tensor("outs", [N, C], f16, kind="ExternalOutput")
    oce = nc.dram_tensor("oce", [OCER, C], f16, kind="ExternalOutput")
    lstall = nc.dram_tensor("lstall", [LBTOT, 1], f32, kind="ExternalOutput")

    # ---------------- compile-time constants ----------------
    ut128_np = (np.arange(128)[:, None] < np.arange(128)[None, :]).astype(np.float32)
    iota_np = (np.arange(NT)[None, :] * 128 + np.arange(128)[:, None]).astype(
        np.float32
    )
    fill_np = np.full((128, 40), TRASH_T, np.float32)
    ident_np = np.zeros((128, 256), dtype=np.float16)
    ident_np[:, :128] = np.eye(128, dtype=np.float16)
    # (c,s)-flattened strict-upper tile-prefix mask: contributes t_(c',s) to
    # (c,s) iff same slot and c' < c
    cs_c = np.arange(128) // E
    cs_s = np.arange(128) % E
    utcs_np = (
        (cs_s[:, None] == cs_s[None, :]) & (cs_c[:, None] < cs_c[None, :])
    ).astype(np.float32)
    lbrow_np = np.array([lbase[s] for s in cs_s], np.float32)[None, :]
    ut128_d = nc.inline_tensor(ut128_np, "ut128c")
    utcs_d = nc.inline_tensor(utcs_np, "utcsc")
    lbrow_d = nc.inline_tensor(lbrow_np, "lbrowc")
    iota_d = nc.inline_tensor(iota_np, "iotac")
    fill_d = nc.inline_tensor(fill_np, "fillc")
    ident_d = nc.inline_tensor(ident_np, "identc")
    ones128_d = nc.inline_tensor(np.ones((128, 1), np.float32), "ones128c")
    onesrow_d = nc.inline_tensor(np.ones((1, 128), np.float32), "onesrowc")

    with TileContext(nc) as tc:
        with (
            tc.tile_pool(name="cpool", bufs=1) as cp,
            tc.tile_pool(name="mpool", bufs=1) as mp,
            tc.tile_pool(name="spool", bufs=1) as sp,
        ):
            # ---- constants into SBUF (scalar queue; sync stays clear) ----
            rw1_sb = cp.tile([128, CT, HR], f32, name="rw1_sb")
            nc.scalar.dma_start(out=rw1_sb[:], in_=rw1[:, :, :])
            ut128 = cp.tile([128, 128], f32, name="ut128")
            nc.scalar.dma_start(out=ut128[:], in_=ut128_d[:, :])
            utcs = cp.tile([128, 128], f32, name="utcs")
            nc.scalar.dma_start(out=utcs[:], in_=utcs_d[:, :])
            lbrow = cp.tile([1, 128], f32, name="lbrow")
            nc.scalar.dma_start(out=lbrow[:], in_=lbrow_d[:, :])
            iota = cp.tile([128, NT], f32, name="iota")
            nc.scalar.dma_start(out=iota[:], in_=iota_d[:, :])
            fill = cp.tile([128, 40], f32, name="fill")
            nc.scalar.dma_start(out=fill[:], in_=fill_d[:, :])
            ident = cp.tile([128, 256], f16, name="ident")
            nc.scalar.dma_start(out=ident[:], in_=ident_d[:, :])
            ones128 = cp.tile([128, 1], f32, name="ones128")
            nc.scalar.dma_start(out=ones128[:], in_=ones128_d[:, :])
            onesrow = cp.tile([1, 128], f32, name="onesrow")
            nc.scalar.dma_start(out=onesrow[:], in_=onesrow_d[:, :])
            rb1_sb = cp.tile([128, HR // 128], f32, name="rb1_sb")
            nc.scalar.dma_start(
                out=rb1_sb[:], in_=rb1.rearrange("(a p) -> p a", p=128)
            )
            rw2_sb = cp.tile([128, HR // 128, E], f32, name="rw2_sb")
            nc.scalar.dma_start(
                out=rw2_sb[:], in_=rw2.rearrange("(a p) e -> p a e", p=128)
            )
            rb2_row = cp.tile([1, E], f32, name="rb2_row")
            nc.scalar.dma_start(out=rb2_row[:], in_=rb2[None, :])
            sb1_sb = cp.tile([128, SSH // 128], f32, name="sb1_sb")
            nc.scalar.dma_start(
                out=sb1_sb[:], in_=sb1.rearrange("(a p) -> p a", p=128)
            )
            b1_sb = cp.tile([128, E, W // 128], f32, name="b1_sb")
            nc.scalar.dma_start(
                out=b1_sb[:], in_=b1s.rearrange("s (a p) -> p s a", p=128)
            )

            wall = cp.tile([128, NT, E], f32, name="wall")
            wall1 = cp.tile([128, NT, E], f32, name="wall1")
            wall2 = cp.tile([128, NT, E], f32, name="wall2")
            toki_all = cp.tile([128, LBTOT // 128], i32, name="toki_all")

            # shared-expert tiles allocated up-front (addresses distinct from
            # the router pool so their DMAs are not blocked by address reuse)
            hs_sb = sp.tile([128, SSH // 128, N], f16, name="hs_sb")
            sw1_sb = sp.tile([128, CT, SSH], f16, name="sw1_sb")
            sw2_sb = sp.tile([128, SSH // 128, C], f16, name="sw2_sb")
            xh_t = [
                sp.tile([128, CT, 512], f16, name=f"xh{g}", tag=f"xh{g}")
                for g in range(NG)
            ]

            # ---- phase R: router in exact fp32 ----
            with (
                tc.tile_pool(name="rpool", bufs=1) as rp,
                tc.tile_pool(name="rpp", bufs=1, space="PSUM") as pp,
            ):
                hr_sb = rp.tile([128, HR // 128, N], f32, name="hr_sb")
                xg_t = []
                for g in range(NG):
                    xt_g = rp.tile([128, CT, 512], f32, name=f"xt{g}", tag=f"xt{g}")
                    nc.sync.dma_start(out=xt_g[:], in_=xtg[g, :, :, :])
                    xg_t.append(xt_g)
                # shared-expert loads queue right behind the router's
                nc.sync.dma_start(out=sw1_sb[:], in_=sw1[:, :, :])
                nc.sync.dma_start(out=sw2_sb[:], in_=sw2[:, :, :])
                for g in range(NG):
                    nc.sync.dma_start(out=xh_t[g][:], in_=xhg[g, :, :, :])

                for g in range(NG):
                    for ht in range(HR // 128):
                        ps_h = pp.tile(
                            [128, 512], f32, name="ps_big", tag="ps_big", bufs=4
                        )
                        for ct in range(CT):
                            nc.tensor.matmul(
                                out=ps_h[:],
                                lhsT=rw1_sb[:, ct, ht * 128 : (ht + 1) * 128],
                                rhs=xg_t[g][:, ct, :],
                                start=(ct == 0),
                                stop=(ct == CT - 1),
                            )
                        nc.scalar.activation(
                            out=hr_sb[:, ht, g * 512 : (g + 1) * 512],
                            in_=ps_h[:],
                            func=AF.Relu,
                            bias=rb1_sb[:, ht : ht + 1],
                        )

                # router L2 + top-2-on-logits epilogue (4 psum tiles deep)
                for tt in range(NT):
                    tok = slice(tt * 128, (tt + 1) * 128)
                    ps_l = pp.tile([128, E], f32, name="ps_l", tag="ps_l", bufs=4)
                    for ht in range(HR // 128):
                        nc.tensor.matmul(
                            out=ps_l[:],
                            lhsT=hr_sb[:, ht, tok],
                            rhs=rw2_sb[:, ht, :],
                            start=(ht == 0),
                            stop=False,
                        )
                    nc.tensor.matmul(
                        out=ps_l[:],
                        lhsT=onesrow[:],
                        rhs=rb2_row[:],
                        start=False,
                        stop=True,
                    )
                    lg = mp.tile([128, E], f32, name="lg", tag="lg", bufs=4)
                    nc.scalar.copy(out=lg[:], in_=ps_l[:])
                    mxl = mp.tile([128, 8], f32, name="mxl", tag="mxl", bufs=4)
                    nc.vector.max(out=mxl[:], in_=lg[:])
                    nc.vector.tensor_scalar(
                        wall[:, tt, :], lg[:], mxl[:, 1:2], None, op0=ALU.is_ge
                    )
                    nc.vector.tensor_scalar(
                        wall1[:, tt, :], lg[:], mxl[:, 0:1], None, op0=ALU.is_ge
                    )

            # ---- compaction: batched prefix (all slots at once), then 32
            # scatters (2 choices x 16 tiles) over 4 independent chains ----
            cpp_ctx = tc.tile_pool(name="cpp", bufs=1, space="PSUM")
            pp = cpp_ctx.__enter__()
            if do_compact:
                nc.vector.tensor_sub(wall2[:, :, :], wall[:, :, :], wall1[:, :, :])
                ps_pre = pp.tile([128, NT, E], f32, name="ps_pre", tag="ps_pre")
                nc.tensor.matmul(
                    out=ps_pre[:, :, :], lhsT=ut128[:], rhs=wall[:, :, :],
                    start=True, stop=False,
                )
                ps_tot = pp.tile([128, 1], f32, name="ps_tot", tag="ps_tot")
                nc.tensor.matmul(
                    out=ps_tot[:], lhsT=wall[:, :, :], rhs=ones128[:],
                    start=True, stop=True,
                )
                tot_sb = mp.tile([128, 1], f32, name="tot_sb")
                nc.vector.tensor_copy(out=tot_sb[:], in_=ps_tot[:])
                ps_pt = pp.tile([1, 128], f32, name="ps_pt", tag="ps_pt")
                nc.tensor.matmul(
                    out=ps_pt[:], lhsT=tot_sb[:], rhs=utcs[:],
                    start=True, stop=True,
                )
                ptot_row = mp.tile([1, 128], f32, name="ptot_row")
                nc.vector.tensor_add(ptot_row[:], ps_pt[:], lbrow[:])
                nc.tensor.matmul(
                    out=ps_pre[:, :, :], lhsT=onesrow[:], rhs=ptot_row[:],
                    start=False, stop=True,
                )
                tmp1 = mp.tile([128, NT, E], f32, name="tmp1")
                nc.vector.tensor_mul(tmp1[:], ps_pre[:, :, :], wall1[:, :, :])
                pos1 = mp.tile([128, NT, 1], f32, name="pos1")
                nc.vector.tensor_reduce(
                    out=pos1[:], in_=tmp1[:], axis=mybir.AxisListType.X, op=ALU.add
                )
                tmp2 = mp.tile([128, NT, E], f32, name="tmp2")
                nc.vector.tensor_mul(tmp2[:], ps_pre[:, :, :], wall2[:, :, :])
                pos2 = mp.tile([128, NT, 1], f32, name="pos2")
                nc.vector.tensor_reduce(
                    out=pos2[:], in_=tmp2[:], axis=mybir.AxisListType.X, op=ALU.add
                )
                # guard: a token with no top-2 entry (exact logit tie) must
                # scatter out of bounds instead of to row 0
                cnt2 = mp.tile([128, NT, 1], f32, name="cnt2")
                nc.vector.tensor_reduce(
                    out=cnt2[:], in_=wall2[:, :, :], axis=mybir.AxisListType.X,
                    op=ALU.add,
                )
                guard = mp.tile([128, NT, 1], f32, name="guard")
                nc.vector.tensor_scalar(
                    guard[:], cnt2[:], -1.0e9, 1.0e9, op0=ALU.mult, op1=ALU.add
                )
                nc.vector.tensor_add(pos2[:], pos2[:], guard[:])
                # transform positions to p-major list rows so the list
                # readback is 128 contiguous runs: row = (pos%128)*NTL+pos//128
                NTL = LBTOT // 128
                posis = []
                for nmo, possrc in (("posi1", pos1), ("posi2", pos2)):
                    pint = mp.tile([128, NT], i32, name=f"{nmo}_i",
                                   tag=f"{nmo}_i")
                    nc.vector.tensor_copy(out=pint[:], in_=possrc[:, :, 0])
                    hi = mp.tile([128, NT], i32, name=f"{nmo}_h",
                                 tag=f"{nmo}_h")
                    nc.vector.tensor_scalar(
                        hi[:], pint[:], 7, None, op0=ALU.logical_shift_right
                    )
                    lo = mp.tile([128, NT], i32, name=f"{nmo}_l",
                                 tag=f"{nmo}_l")
                    nc.vector.tensor_scalar(
                        lo[:], pint[:], 127, None, op0=ALU.bitwise_and
                    )
                    nc.vector.tensor_scalar(
                        lo[:], lo[:], NTL, None, op0=ALU.mult
                    )
                    posi = mp.tile([128, NT], i32, name=nmo, tag=nmo)
                    nc.vector.tensor_add(posi[:], lo[:], hi[:])
                    posis.append(posi)
                posi1, posi2 = posis
                NCH = 8
                with tc.tile_pool(name="dpool", bufs=1, space="DRAM") as dp:
                    lsts = []
                    for k in range(NCH):
                        lk = dp.tile([LBTOT, 1], f32, name=f"lst{k}",
                                     tag=f"lst{k}")
                        lsts.append(lk)
                        # p-major fill AP: 144B-contiguous runs, cheap to
                        # generate; the layout is irrelevant for a constant
                        nc.scalar.dma_start(
                            out=lk[:, :].rearrange("(p a) x -> p (a x)", p=128),
                            in_=fill[:, : LBTOT // 128],
                        )
                    for c in range(NT):
                        for ch, posi in (
                            (c % (NCH // 2), posi1),
                            (NCH // 2 + c % (NCH // 2), posi2),
                        ):
                            nc.gpsimd.indirect_dma_start(
                                out=lsts[ch][:, :],
                                out_offset=bass.IndirectOffsetOnAxis(
                                    ap=posi[:, c : c + 1], axis=0
                                ),
                                in_=iota[:, c : c + 1],
                                in_offset=None,
                                bounds_check=LBTOT - 1,
                                oob_is_err=False,
                            )
                    # read back the lists (spread over the three DMA queues),
                    # merge: written rows hold tok<2048, unwritten hold 2048
                    # -> sum - (NCH-1)*2048 is the union
                    iws = []
                    for k in range(NCH):
                        iwk = mp.tile([128, LBTOT // 128], f32, name=f"iw{k}",
                                      tag=f"iw{k}")
                        iws.append(iwk)
                        nc.gpsimd.dma_start(
                            out=iwk[:, :],
                            in_=lsts[k][:, :].rearrange(
                                "(p a) x -> p (a x)", p=128
                            ),
                        )
                    iwm = mp.tile([128, LBTOT // 128], f32, name="iwm")
                    nc.vector.tensor_add(iwm[:], iws[0][:], iws[1][:])
                    for k in range(2, NCH):
                        nc.vector.tensor_add(iwm[:], iwm[:], iws[k][:])
                    nc.vector.tensor_scalar_add(iwm[:], iwm[:], -(NCH - 1.0) * N)
                    nc.vector.tensor_copy(out=toki_all[:, :], in_=iwm[:])
                    nc.sync.dma_start(
                        out=lstall.rearrange("(p a) x -> p (a x)", p=128),
                        in_=iwm[:, :],
                    )
            cpp_ctx.__exit__(None, None, None)

            # ---- shared expert (fp16, I-slice): fills the scatter/gather
            # latency; then expert slots ----
            with tc.tile_pool(name="spp", bufs=1, space="PSUM") as pp:
                for g in (range(NG) if do_shared else []):
                    for it in range(SSH // 128):
                        ps_s = pp.tile(
                            [128, 512], f32, name="ps_big2", tag="ps_big", bufs=4
                        )
                        for ct in range(CT):
                            nc.tensor.matmul(
                                out=ps_s[:],
                                lhsT=sw1_sb[:, ct, it * 128 : (it + 1) * 128],
                                rhs=xh_t[g][:, ct, :],
                                start=(ct == 0),
                                stop=(ct == CT - 1),
                            )
                        nc.scalar.activation(
                            out=hs_sb[:, it, g * 512 : (g + 1) * 512],
                            in_=ps_s[:],
                            func=AF.Silu,
                            bias=sb1_sb[:, it : it + 1],
                        )

                def shared_l2(tiles):
                    for tt in tiles:
                        tok = slice(tt * 128, (tt + 1) * 128)
                        orow = sp.tile([128, C], f16, name="sorow", tag="sorow",
                                       bufs=3)
                        for hh in range(2):
                            csl = slice(hh * 512, (hh + 1) * 512)
                            ps2 = pp.tile([128, 512], f32, name="ps_big3",
                                          tag="ps_big", bufs=4)
                            for it in range(SSH // 128):
                                nc.tensor.matmul(
                                    out=ps2[:],
                                    lhsT=hs_sb[:, it, tok],
                                    rhs=sw2_sb[:, it, csl],
                                    start=(it == 0),
                                    stop=(it == SSH // 128 - 1),
                                )
                            nc.vector.tensor_copy(out=orow[:, csl], in_=ps2[:])
                        nc.scalar.dma_start(out=outs[tok, :], in_=orow[:])

                # ---- expert slots (ascending cap order); the tail of the
                # shared expert is emitted after the first gathers so the PE
                # has work while the scatter/gather chain drains ----
                with tc.tile_pool(name="epool", bufs=1) as ep:
                    capmax = max(caps)
                    lbmax = max(lbs)

                    def load_w(s):
                        w1t = ep.tile([128, CT, W], f16, name="w1t", tag="w1t",
                                      bufs=2)
                        nc.sync.dma_start(out=w1t[:], in_=w1s[s, :, :, :])
                        w2t = ep.tile([128, W // 128, C], f16, name="w2t",
                                      tag="w2t", bufs=2)
                        nc.sync.dma_start(out=w2t[:], in_=w2s[s, :, :, :])
                        return w1t, w2t

                    def emit_gathers(s):
                        tiles = []
                        lbcol = int(lbase[s]) // 128
                        for r in range(lbs[s] // 128):
                            xgr = ep.tile([128, C], f16, name="xgr", tag="xgr",
                                          bufs=6)
                            nc.gpsimd.indirect_dma_start(
                                out=xgr[:, :],
                                out_offset=None,
                                in_=xp[:, :],
                                in_offset=bass.IndirectOffsetOnAxis(
                                    ap=toki_all[:, lbcol + r : lbcol + r + 1],
                                    axis=0,
                                ),
                            )
                            tiles.append(xgr)
                        return tiles

                    wpre = {}
                    xgr_pre = {}
                    if do_expert:
                        for s in sorder[:2]:
                            wpre[s] = load_w(s)
                    if do_shared:
                        shared_l2(range(8))
                    if do_expert:
                        for s in sorder[:2]:
                            xgr_pre[s] = emit_gathers(s)
                    if do_shared:
                        shared_l2(range(8, NT))
                    for s in (sorder if do_expert else []):
                        cap, lb = caps[s], lbs[s]
                        ntile = lb // 128
                        w1t, w2t = wpre[s] if s in wpre else load_w(s)
                        xgrs = xgr_pre[s] if s in xgr_pre else emit_gathers(s)
                        xgt = ep.tile([128, CT, lbmax], f16, name="xgt",
                                      tag="xgt", bufs=1)
                        for r in range(ntile):
                            xgr = xgrs[r]
                            for kk in range(CT // 4):
                                ps_t = pp.tile([128, 512], f16, name="ps_tr",
                                               tag="ps_tr", bufs=3)
                                for j in range(4):
                                    ct = kk * 4 + j
                                    nc.tensor.transpose(
                                        out=ps_t[:, j * 128 : (j + 1) * 128],
                                        in_=xgr[:, ct * 128 : (ct + 1) * 128],
                                        identity=ident[:, :128],
                                    )
                                nc.vector.tensor_copy(
                                    out=xgt[
                                        :, kk * 4 : (kk + 1) * 4,
                                        r * 128 : (r + 1) * 128,
                                    ],
                                    in_=ps_t[:],
                                )
                        # L1: h^T = silu(W1^T @ Xg^T + b1)
                        hq = ep.tile([128, W // 128, capmax], f16, name="hq",
                                     tag="hq", bufs=1)
                        for it in range(W // 128):
                            for g0 in range(0, cap, 512):
                                gn = min(512, cap - g0)
                                ps1 = pp.tile([128, 512], f32, name="ps_e1",
                                              tag="ps_big", bufs=4)
                                for ct in range(CT):
                                    nc.tensor.matmul(
                                        out=ps1[:, :gn],
                                        lhsT=w1t[:, ct, it * 128 : (it + 1) * 128],
                                        rhs=xgt[:, ct, g0 : g0 + gn],
                                        start=(ct == 0),
                                        stop=(ct == CT - 1),
                                    )
                                nc.scalar.activation(
                                    out=hq[:, it, g0 : g0 + gn],
                                    in_=ps1[:, :gn],
                                    func=AF.Silu,
                                    bias=b1_sb[:, s, it : it + 1],
                                )
                        # L2: compact output rows (no gate scale, host does it)
                        for t0 in range(0, cap, 128):
                            tn = min(128, cap - t0)
                            orow = ep.tile([128, C], f16, name="eor", tag="eor",
                                           bufs=4)
                            for hh in range(2):
                                csl = slice(hh * 512, (hh + 1) * 512)
                                ps2 = pp.tile([128, 512], f32, name="ps_e2",
                                              tag="ps_big", bufs=4)
                                for it in range(W // 128):
                                    nc.tensor.matmul(
                                        out=ps2[:tn, :],
                                        lhsT=hq[:, it, t0 : t0 + tn],
                                        rhs=w2t[:, it, csl],
                                        start=(it == 0),
                                        stop=(it == W // 128 - 1),
                                    )
                                nc.vector.tensor_copy(
                                    out=orow[:tn, csl], in_=ps2[:tn, :]
                                )
                            nc.scalar.dma_start(
                                out=oce[
                                    int(ocebase[s]) + t0 : int(ocebase[s])
                                    + t0 + tn,
                                    :,
                                ],
                                in_=orow[:tn, :],
                            )

    nc.finalize()
    _BUILD_CACHE[key] = (nc, lbs, ocebase)
    return _BUILD_CACHE[key]


def _pmaj(a):
    """[R, F] -> [128, R//128, F] partition-major slab (large DMA runs)."""
    r, f = a.shape
    return np.ascontiguousarray(a.reshape(r // 128, 128, f).transpose(1, 0, 2))


def _make_in_maps(inputs, p):
    x = np.ascontiguousarray(np.asarray(inputs["x"], np.float32).reshape(N, C))
    xt = x.T                                              # [C, N]
    xtg_np = np.stack(
        [_pmaj(np.ascontiguousarray(xt[:, g * 512 : (g + 1) * 512]))
         for g in range(NG)]
    )
    xth = xt.astype(np.float16)
    xhg_np = np.stack(
        [_pmaj(np.ascontiguousarray(xth[:, g * 512 : (g + 1) * 512]))
         for g in range(NG)]
    )
    xp_np = np.zeros((XPAD, C), np.float16)
    xp_np[:N] = x.astype(np.float16)
    ew1 = np.asarray(inputs["ew1"])
    eb1 = np.asarray(inputs["eb1"])
    ew2 = np.asarray(inputs["ew2"])
    sw1_np = np.asarray(inputs["sw1"])
    sw2_np = np.asarray(inputs["sw2"])
    sb1_np = np.asarray(inputs["sb1"])
    rw1p_np = _pmaj(np.asarray(inputs["rw1"], np.float32))

    in_maps = []
    for c in range(NCORES):
        isl = slice(c * W, (c + 1) * W)
        w1sp = np.stack(
            [_pmaj(ew1[e][:, isl].astype(np.float16)) for e in range(E)]
        )
        w2sp = np.stack(
            [_pmaj(np.ascontiguousarray(ew2[e][isl, :]).astype(np.float16))
             for e in range(E)]
        )
        in_maps.append(
            {
                "xtg": xtg_np,
                "xhg": xhg_np,
                "xp": xp_np,
                "rw1p": rw1p_np,
                "rb1": np.asarray(inputs["rb1"], np.float32),
                "rw2": np.asarray(inputs["rw2"], np.float32),
                "rb2": np.asarray(inputs["rb2"], np.float32),
                "sw1p": _pmaj(sw1_np[:, isl].astype(np.float16)),
                "sb1s": np.ascontiguousarray(sb1_np[isl].astype(np.float32)),
                "sw2p": _pmaj(
                    np.ascontiguousarray(sw2_np[isl, :]).astype(np.float16)
                ),
                "w1sp": w1sp,
                "b1s": np.ascontiguousarray(eb1[:, isl].astype(np.float32)),
                "w2sp": w2sp,
            }
        )
    return in_maps


def run_spmd(inputs, **kw):
    p = plan(inputs)
    nc, lbs, ocebase = build_nc(p["caps"])
    in_maps = _make_in_maps(inputs, p)
    res = run_bass_kernel_spmd(nc, in_maps, core_ids=list(range(NCORES)), **kw)
    return res, p


def kernel(**inputs) -> np.ndarray:
    p = plan(inputs)
    res, _ = run_spmd(inputs)
    caps = p["caps"]
    gates = p["gates"]                                # [N, E] fp64 softmax
    eb2 = np.asarray(inputs["eb2"], np.float64)       # [E, C]
    sb2 = np.asarray(inputs["sb2"], np.float64)       # [C]

    acc = np.zeros((N, C), np.float64)
    for c in range(NCORES):
        acc += res.results[c]["outs"].astype(np.float64)
    acc += sb2[None, :]

    ocesum = np.zeros((sum(caps), C), np.float64)
    for c in range(NCORES):
        ocesum += res.results[c]["oce"].astype(np.float64)

    # device token lists (identical on every core; use core 0)
    lbs = [_ceil(cap, 128) for cap in caps]
    lbase = np.cumsum([0] + lbs)
    # lstall rows are p-major: row p*NTL+a holds the token at pos a*128+p
    lrows_all = (
        np.asarray(res.results[0]["lstall"]).reshape(128, -1).T.reshape(-1)
    )
    base = 0
    slot_toks, slot_rows = [], []
    sel = np.zeros((N, E), np.float64)
    for s in range(E):
        toks = lrows_all[lbase[s] : lbase[s] + caps[s]].astype(np.int64)
        valid = toks < N
        slot_toks.append(toks[valid])
        rows = ocesum[base : base + caps[s]][valid]
        slot_rows.append(rows)
        sel[toks[valid], s] = 1.0
        base += caps[s]

    # combine weights: softmax(top-k gates / TOPK) over the selected pair
    wexp = np.exp(gates / TOPK) * sel
    wsum = wexp.sum(-1, keepdims=True)
    wsum[wsum == 0] = 1.0
    wn = wexp / wsum
    for s in range(E):
        t = slot_toks[s]
        acc[t] += wn[t, s][:, None] * (slot_rows[s] + eb2[s][None, :])

    return acc.astype(np.float32).reshape(B, T, C)


# revision 33
# speedup vs baseline: 1.2904x; 1.0341x over previous
"""MoE (top-2 of 8 experts, shared expert) Trainium2 Bass kernel, 8-core SPMD.

Strategy (expert parallelism, I-sliced for perfect balance):
 - Slot s on every core processes expert s restricted to the core's I-slice
   [c*512:(c+1)*512].  Every core therefore runs ALL experts on identical
   token counts -> per-core work is equal by construction, and each expert
   weight is loaded exactly once across the machine (fp16, 16MB/core).
 - Router is replicated in exact fp32; the top-2 SELECTION is done on
   LOGITS (exact matmul output), not on softmax gates, so the noisy exp
   activation cannot flip the selection.  The device only produces the
   top-2 mask + compact per-slot token lists; softmax gate values, expert
   biases and the final combine are applied on the host in fp64 from the
   device token lists.
 - Compaction: one batched prefix-sum (3 matmuls for all 8 slots at once)
   computes a global list position for every token's top-1 and top-2
   choice; 32 indirect scatters (128 offsets each, the HW limit) spread
   over 4 independent DRAM lists pipeline on the gpsimd queue; lists are
   merged, written to the lstall output, and reused as gather offsets.
 - Expert FFN in fp16: per-row-tile gathers -> PE transposes (4 per PSUM
   bank) -> L1 silu -> L2 -> compact fp16 output rows.
 - Shared expert is I-sliced 8 ways in fp16 and placed after the
   compaction chains so its matmuls cover the scatter/gather latency.
 - All bulk inputs are shipped partition-major ([128, ct, free] slabs) so
   each DMA descriptor covers 8-16KB; descriptor generation (~11.5ns/desc)
   otherwise caps a queue at ~100GB/s.
 - Host unshards: sums the 8 shared partials and 8 expert partials (the
   I-slices of a token's expert rows add up to the full FFN), applies
   host-softmax gates re-normalized over the device-selected pair.
"""

import os
import sys

sys.path.insert(0, "/opt/trn_rl_repo")

import numpy as np

import concourse.bass as bass
import concourse.mybir as mybir
from concourse import bacc
from concourse.tile import TileContext
from concourse.bass_utils import run_bass_kernel_spmd

f32 = mybir.dt.float32
f16 = mybir.dt.float16
i32 = mybir.dt.int32
AF = mybir.ActivationFunctionType
ALU = mybir.AluOpType

B, T, C, I, E, TOPK = 2, 1024, 1024, 4096, 8, 2
N = B * T                     # 2048 tokens
NCORES = 8
W = I // NCORES               # expert I-slice width per core (512)
SSH = I // NCORES             # shared-expert I-slice width (512)
HR = C // 4                   # router hidden (256)
XPAD = N + 128                # padded token rows; rows >= 2048 are zeros
TRASH_T = float(N)            # trash token id (gathers zeros)
NT = N // 128                 # 16 token tiles
CT = C // 128                 # 8 contraction tiles
NG = N // 512                 # 4 token groups

_BUILD_CACHE = {}


def _ceil(x, g):
    return -(-x // g) * g


def plan(inputs):
    """Host-side routing estimate: exact caps + gates for the combine.

    The fp32 numpy router matches the device fp32 router selection: the
    smallest top2-vs-top3 logit gap for this input is ~1.3e-4, vastly
    above both computations' noise.
    """
    x = np.asarray(inputs["x"], np.float32).reshape(N, C)
    h = np.maximum(x @ np.asarray(inputs["rw1"]) + np.asarray(inputs["rb1"]), 0)
    logits = (h @ np.asarray(inputs["rw2"]) + np.asarray(inputs["rb2"])).astype(
        np.float64
    )
    g = np.exp(logits - logits.max(-1, keepdims=True))
    g /= g.sum(-1, keepdims=True)
    top2 = np.argsort(-logits, axis=-1)[:, :TOPK]
    counts = np.bincount(top2.ravel(), minlength=E)
    caps = [max(128, _ceil(int(c) + 4, 64)) for c in counts]
    return {"caps": caps, "counts": counts, "gates": g}


def build_nc(caps):
    key = (tuple(caps), os.environ.get("MOE_STOP", ""))
    if key in _BUILD_CACHE:
        return _BUILD_CACHE[key]

    lbs = [_ceil(cap, 128) for cap in caps]          # list/gather rows per slot
    ocebase = np.cumsum([0] + caps)                  # oce row offsets
    OCER = int(ocebase[-1])
    lbase = np.cumsum([0] + lbs)                     # list section offsets
    LBTOT = int(lbase[-1])
    # expert slot processing order: small caps first so weight DMA keeps up
    sorder = sorted(range(E), key=lambda s: caps[s])

    stop = os.environ.get("MOE_STOP", "")
    do_compact = stop != "router"
    do_shared = do_compact and stop != "compact"
    do_expert = do_shared and stop != "shared"

    nc = bacc.Bacc("TRN2", target_bir_lowering=False)

    # ---------------- I/O (bulk tensors partition-major) ----------------
    xhg = nc.dram_tensor("xhg", [NG, 128, CT, 512], f16, kind="ExternalInput")
    xlg = nc.dram_tensor("xlg", [NG, 128, CT, 512], f16, kind="ExternalInput")
    xp = nc.dram_tensor("xp", [XPAD, C], f16, kind="ExternalInput")
    rw1h = nc.dram_tensor("rw1h", [128, CT, HR], f16, kind="ExternalInput")
    rw1l = nc.dram_tensor("rw1l", [128, CT, HR], f16, kind="ExternalInput")
    rb1 = nc.dram_tensor("rb1", [HR], f32, kind="ExternalInput")
    rw2 = nc.dram_tensor("rw2", [HR, E], f32, kind="ExternalInput")
    rb2 = nc.dram_tensor("rb2", [E], f32, kind="ExternalInput")
    sw1 = nc.dram_tensor("sw1p", [128, CT, SSH], f16, kind="ExternalInput")
    sb1 = nc.dram_tensor("sb1s", [SSH], f32, kind="ExternalInput")
    sw2 = nc.dram_tensor("sw2p", [128, SSH // 128, C], f16, kind="ExternalInput")
    w1s = nc.dram_tensor("w1sp", [E, 128, CT, W], f16, kind="ExternalInput")
    b1s = nc.dram_tensor("b1s", [E, W], f32, kind="ExternalInput")
    w2s = nc.dram_tensor("w2sp", [E, 128, W // 128, C], f16, kind="ExternalInput")

    outs = nc.dram_tensor("outs", [N, C], f16, kind="ExternalOutput")
    oce = nc.dram_tensor("oce", [OCER, C], f16, kind="ExternalOutput")
    lstall = nc.dram_tensor("lstall", [LBTOT, 1], f32, kind="ExternalOutput")

    # ---------------- compile-time constants ----------------
    ut128_np = (np.arange(128)[:, None] < np.arange(128)[None, :]).astype(np.float32)
    iota_np = (np.arange(NT)[None, :] * 128 + np.arange(128)[:, None]).astype(
        np.float32
    )
    fill_np = np.full((128, 40), TRASH_T, np.float32)
    ident_np = np.zeros((128, 256), dtype=np.float16)
    ident_np[:, :128] = np.eye(128, dtype=np.float16)
    # (c,s)-flattened strict-upper tile-prefix mask: contributes t_(c',s) to
    # (c,s) iff same slot and c' < c
    cs_c = np.arange(128) // E
    cs_s = np.arange(128) % E
    utcs_np = (
        (cs_s[:, None] == cs_s[None, :]) & (cs_c[:, None] < cs_c[None, :])
    ).astype(np.float32)
    lbrow_np = np.array([lbase[s] for s in cs_s], np.float32)[None, :]
    ut128_d = nc.inline_tensor(ut128_np, "ut128c")
    utcs_d = nc.inline_tensor(utcs_np, "utcsc")
    lbrow_d = nc.inline_tensor(lbrow_np, "lbrowc")
    iota_d = nc.inline_tensor(iota_np, "iotac")
    fill_d = nc.inline_tensor(fill_np, "fillc")
    ident_d = nc.inline_tensor(ident_np, "identc")
    ones128_d = nc.inline_tensor(np.ones((128, 1), np.float32), "ones128c")
    onesrow_d = nc.inline_tensor(np.ones((1, 128), np.float32), "onesrowc")

    with TileContext(nc) as tc:
        with (
            tc.tile_pool(name="cpool", bufs=1) as cp,
            tc.tile_pool(name="mpool", bufs=1) as mp,
            tc.tile_pool(name="spool", bufs=1) as sp,
        ):
            # ---- constants into SBUF (scalar queue; sync stays clear) ----
            rw1h_sb = cp.tile([128, CT, HR], f16, name="rw1h_sb")
            nc.scalar.dma_start(out=rw1h_sb[:], in_=rw1h[:, :, :])
            rw1l_sb = cp.tile([128, CT, HR], f16, name="rw1l_sb")
            nc.scalar.dma_start(out=rw1l_sb[:], in_=rw1l[:, :, :])
            ut128 = cp.tile([128, 128], f32, name="ut128")
            nc.scalar.dma_start(out=ut128[:], in_=ut128_d[:, :])
            utcs = cp.tile([128, 128], f32, name="utcs")
            nc.scalar.dma_start(out=utcs[:], in_=utcs_d[:, :])
            lbrow = cp.tile([1, 128], f32, name="lbrow")
            nc.scalar.dma_start(out=lbrow[:], in_=lbrow_d[:, :])
            iota = cp.tile([128, NT], f32, name="iota")
            nc.scalar.dma_start(out=iota[:], in_=iota_d[:, :])
            fill = cp.tile([128, 40], f32, name="fill")
            nc.scalar.dma_start(out=fill[:], in_=fill_d[:, :])
            ident = cp.tile([128, 256], f16, name="ident")
            nc.scalar.dma_start(out=ident[:], in_=ident_d[:, :])
            ones128 = cp.tile([128, 1], f32, name="ones128")
            nc.scalar.dma_start(out=ones128[:], in_=ones128_d[:, :])
            onesrow = cp.tile([1, 128], f32, name="onesrow")
            nc.scalar.dma_start(out=onesrow[:], in_=onesrow_d[:, :])
            rb1_sb = cp.tile([128, HR // 128], f32, name="rb1_sb")
            nc.scalar.dma_start(
                out=rb1_sb[:], in_=rb1.rearrange("(a p) -> p a", p=128)
            )
            rw2_sb = cp.tile([128, HR // 128, E], f32, name="rw2_sb")
            nc.scalar.dma_start(
                out=rw2_sb[:], in_=rw2.rearrange("(a p) e -> p a e", p=128)
            )
            rb2_row = cp.tile([1, E], f32, name="rb2_row")
            nc.scalar.dma_start(out=rb2_row[:], in_=rb2[None, :])
            sb1_sb = cp.tile([128, SSH // 128], f32, name="sb1_sb")
            nc.scalar.dma_start(
                out=sb1_sb[:], in_=sb1.rearrange("(a p) -> p a", p=128)
            )
            b1_sb = cp.tile([128, E, W // 128], f32, name="b1_sb")
            nc.scalar.dma_start(
                out=b1_sb[:], in_=b1s.rearrange("s (a p) -> p s a", p=128)
            )

            wall = cp.tile([128, NT, E], f32, name="wall")
            wall1 = cp.tile([128, NT, E], f32, name="wall1")
            wall2 = cp.tile([128, NT, E], f32, name="wall2")
            nc.vector.memset(wall[:], 0.0)
            nc.vector.memset(wall1[:], 0.0)
            toki_all = cp.tile([128, LBTOT // 128], i32, name="toki_all")

            # shared-expert tiles allocated up-front (addresses distinct from
            # the router pool so their DMAs are not blocked by address reuse)
            hs_sb = sp.tile([128, SSH // 128, N], f16, name="hs_sb")
            sw1_sb = sp.tile([128, CT, SSH], f16, name="sw1_sb")
            sw2_sb = sp.tile([128, SSH // 128, C], f16, name="sw2_sb")
            xh_t = [
                sp.tile([128, CT, 512], f16, name=f"xh{g}", tag=f"xh{g}")
                for g in range(NG)
            ]

            # ---- compaction scaffolding (PSUM + DRAM pools coexist with
            # the router pools; scatters start as soon as the first 8
            # epilogue tiles are done) ----
            NCH = 8
            cpp_ctx = tc.tile_pool(name="cpp", bufs=1, space="PSUM")
            cpp = cpp_ctx.__enter__()
            dp_ctx = tc.tile_pool(name="dpool", bufs=1, space="DRAM")
            dp = dp_ctx.__enter__()
            lsts = []
            if do_compact:
                for k in range(NCH):
                    lk = dp.tile([LBTOT, 1], f32, name=f"lst{k}", tag=f"lst{k}")
                    lsts.append(lk)
                    nc.scalar.dma_start(
                        out=lk[:, :].rearrange("(p a) x -> p (a x)", p=128),
                        in_=fill[:, : LBTOT // 128],
                    )
            NTL = LBTOT // 128

            def compact_phase(cols, ph):
                """Batched rank computation on the CURRENT wall (columns not
                yet written are zero), then scatter the given columns.  The
                rank of a token in column c only involves columns <= c, so
                phase A (cols 0-7) is exact before tiles 8-15 exist."""
                nc.vector.tensor_sub(wall2[:, :, :], wall[:, :, :],
                                     wall1[:, :, :])
                ps_pre = cpp.tile([128, NT, E], f32, name=f"ps_pre{ph}",
                                  tag="ps_pre", bufs=1)
                nc.tensor.matmul(
                    out=ps_pre[:, :, :], lhsT=ut128[:], rhs=wall[:, :, :],
                    start=True, stop=False,
                )
                ps_tot = cpp.tile([128, 1], f32, name=f"ps_tot{ph}",
                                  tag="ps_tot", bufs=1)
                nc.tensor.matmul(
                    out=ps_tot[:], lhsT=wall[:, :, :], rhs=ones128[:],
                    start=True, stop=True,
                )
                tot_sb = mp.tile([128, 1], f32, name=f"tot{ph}", tag=f"tot{ph}")
                nc.vector.tensor_copy(out=tot_sb[:], in_=ps_tot[:])
                ps_pt = cpp.tile([1, 128], f32, name=f"ps_pt{ph}", tag="ps_pt",
                                 bufs=1)
                nc.tensor.matmul(
                    out=ps_pt[:], lhsT=tot_sb[:], rhs=utcs[:],
                    start=True, stop=True,
                )
                ptot_row = mp.tile([1, 128], f32, name=f"ptr{ph}", tag=f"ptr{ph}")
                nc.vector.tensor_add(ptot_row[:], ps_pt[:], lbrow[:])
                nc.tensor.matmul(
                    out=ps_pre[:, :, :], lhsT=onesrow[:], rhs=ptot_row[:],
                    start=False, stop=True,
                )
                tmp1 = mp.tile([128, NT, E], f32, name=f"tmp1{ph}",
                               tag=f"tmp1{ph}")
                nc.vector.tensor_mul(tmp1[:], ps_pre[:, :, :], wall1[:, :, :])
                pos1 = mp.tile([128, NT, 1], f32, name=f"pos1{ph}",
                               tag=f"pos1{ph}")
                nc.vector.tensor_reduce(
                    out=pos1[:], in_=tmp1[:], axis=mybir.AxisListType.X,
                    op=ALU.add,
                )
                tmp2 = mp.tile([128, NT, E], f32, name=f"tmp2{ph}",
                               tag=f"tmp2{ph}")
                nc.vector.tensor_mul(tmp2[:], ps_pre[:, :, :], wall2[:, :, :])
                pos2 = mp.tile([128, NT, 1], f32, name=f"pos2{ph}",
                               tag=f"pos2{ph}")
                nc.vector.tensor_reduce(
                    out=pos2[:], in_=tmp2[:], axis=mybir.AxisListType.X,
                    op=ALU.add,
                )
                cnt2 = mp.tile([128, NT, 1], f32, name=f"cnt2{ph}",
                               tag=f"cnt2{ph}")
                nc.vector.tensor_reduce(
                    out=cnt2[:], in_=wall2[:, :, :], axis=mybir.AxisListType.X,
                    op=ALU.add,
                )
                guard = mp.tile([128, NT, 1], f32, name=f"guard{ph}",
                                tag=f"guard{ph}")
                nc.vector.tensor_scalar(
                    guard[:], cnt2[:], -1.0e9, 1.0e9, op0=ALU.mult,
                    op1=ALU.add,
                )
                nc.vector.tensor_add(pos2[:], pos2[:], guard[:])
                posis = []
                for nmo, possrc in ((f"p1{ph}", pos1), (f"p2{ph}", pos2)):
                    pint = mp.tile([128, NT], i32, name=f"{nmo}i",
                                   tag=f"{nmo}i")
                    nc.vector.tensor_copy(out=pint[:], in_=possrc[:, :, 0])
                    hi = mp.tile([128, NT], i32, name=f"{nmo}h", tag=f"{nmo}h")
                    nc.vector.tensor_scalar(
                        hi[:], pint[:], 7, None, op0=ALU.logical_shift_right
                    )
                    lo = mp.tile([128, NT], i32, name=f"{nmo}l", tag=f"{nmo}l")
                    nc.vector.tensor_scalar(
                        lo[:], pint[:], 127, None, op0=ALU.bitwise_and
                    )
                    nc.vector.tensor_scalar(
                        lo[:], lo[:], NTL, None, op0=ALU.mult
                    )
                    posi = mp.tile([128, NT], i32, name=nmo, tag=nmo)
                    nc.vector.tensor_add(posi[:], lo[:], hi[:])
                    posis.append(posi)
                for c in cols:
                    for ch, posi in (
                        (c % (NCH // 2), posis[0]),
                        (NCH // 2 + c % (NCH // 2), posis[1]),
                    ):
                        nc.gpsimd.indirect_dma_start(
                            out=lsts[ch][:, :],
                            out_offset=bass.IndirectOffsetOnAxis(
                                ap=posi[:, c : c + 1], axis=0
                            ),
                            in_=iota[:, c : c + 1],
                            in_offset=None,
                            bounds_check=LBTOT - 1,
                            oob_is_err=False,
                        )

            # ---- phase R: router (fp16 hi/lo pair, fp32-class exact) ----
            with (
                tc.tile_pool(name="rpool", bufs=1) as rp,
                tc.tile_pool(name="rpp", bufs=1, space="PSUM") as pp,
            ):
                hr_sb = rp.tile([128, HR // 128, N], f32, name="hr_sb")
                xl_t = []
                for g in range(NG):
                    nc.sync.dma_start(out=xh_t[g][:], in_=xhg[g, :, :, :])
                    xl_g = rp.tile([128, CT, 512], f16, name=f"xl{g}", tag=f"xl{g}")
                    nc.sync.dma_start(out=xl_g[:], in_=xlg[g, :, :, :])
                    xl_t.append(xl_g)
                # shared-expert weight loads queue right behind the router's
                nc.sync.dma_start(out=sw1_sb[:], in_=sw1[:, :, :])
                nc.sync.dma_start(out=sw2_sb[:], in_=sw2[:, :, :])

                # x = hi + lo (fp16 pair); x@w = hi@w_hi + lo@w_hi + hi@w_lo
                # to fp32-class accuracy at 3 fp16 passes (vs 4 for fp32)
                for g in range(NG):
                    for ht in range(HR // 128):
                        ps_h = pp.tile(
                            [128, 512], f32, name="ps_big", tag="ps_big", bufs=3
                        )
                        hsl = slice(ht * 128, (ht + 1) * 128)
                        for ct in range(CT):
                            nc.tensor.matmul(
                                out=ps_h[:],
                                lhsT=rw1h_sb[:, ct, hsl],
                                rhs=xh_t[g][:, ct, :],
                                start=(ct == 0),
                                stop=False,
                            )
                            nc.tensor.matmul(
                                out=ps_h[:],
                                lhsT=rw1h_sb[:, ct, hsl],
                                rhs=xl_t[g][:, ct, :],
                                start=False,
                                stop=False,
                            )
                            nc.tensor.matmul(
                                out=ps_h[:],
                                lhsT=rw1l_sb[:, ct, hsl],
                                rhs=xh_t[g][:, ct, :],
                                start=False,
                                stop=(ct == CT - 1),
                            )
                        nc.scalar.activation(
                            out=hr_sb[:, ht, g * 512 : (g + 1) * 512],
                            in_=ps_h[:],
                            func=AF.Relu,
                            bias=rb1_sb[:, ht : ht + 1],
                        )

                # router L2 + top-2-on-logits epilogue (4 psum tiles deep)
                for tt in range(NT):
                    tok = slice(tt * 128, (tt + 1) * 128)
                    ps_l = pp.tile([128, E], f32, name="ps_l", tag="ps_l", bufs=2)
                    for ht in range(HR // 128):
                        nc.tensor.matmul(
                            out=ps_l[:],
                            lhsT=hr_sb[:, ht, tok],
                            rhs=rw2_sb[:, ht, :],
                            start=(ht == 0),
                            stop=False,
                        )
                    nc.tensor.matmul(
                        out=ps_l[:],
                        lhsT=onesrow[:],
                        rhs=rb2_row[:],
                        start=False,
                        stop=True,
                    )
                    lg = mp.tile([128, E], f32, name="lg", tag="lg", bufs=4)
                    nc.scalar.copy(out=lg[:], in_=ps_l[:])
                    mxl = mp.tile([128, 8], f32, name="mxl", tag="mxl", bufs=4)
                    nc.vector.max(out=mxl[:], in_=lg[:])
                    nc.vector.tensor_scalar(
                        wall[:, tt, :], lg[:], mxl[:, 1:2], None, op0=ALU.is_ge
                    )
                    nc.vector.tensor_scalar(
                        wall1[:, tt, :], lg[:], mxl[:, 0:1], None, op0=ALU.is_ge
                    )
                    if do_compact and tt == 7:
                        compact_phase(range(0, 8), "A")
                    if do_compact and tt == NT - 1:
                        compact_phase(range(8, NT), "B")

            # ---- list readback + merge -> gather offsets ----
            if do_compact:
                iws = []
                for k in range(NCH):
                    iwk = mp.tile([128, LBTOT // 128], f32, name=f"iw{k}",
                                  tag=f"iw{k}")
                    iws.append(iwk)
                    nc.gpsimd.dma_start(
                        out=iwk[:, :],
                        in_=lsts[k][:, :].rearrange("(p a) x -> p (a x)", p=128),
                    )
                iwm = mp.tile([128, LBTOT // 128], f32, name="iwm")
                nc.vector.tensor_add(iwm[:], iws[0][:], iws[1][:])
                for k in range(2, NCH):
                    nc.vector.tensor_add(iwm[:], iwm[:], iws[k][:])
                nc.vector.tensor_scalar_add(iwm[:], iwm[:], -(NCH - 1.0) * N)
                nc.vector.tensor_copy(out=toki_all[:, :], in_=iwm[:])
                nc.sync.dma_start(
                    out=lstall.rearrange("(p a) x -> p (a x)", p=128),
                    in_=iwm[:, :],
                )
            dp_ctx.__exit__(None, None, None)
            cpp_ctx.__exit__(None, None, None)

            # ---- shared expert (fp16, I-slice): fills the scatter/gather
            # latency; then expert slots ----
            with tc.tile_pool(name="spp", bufs=1, space="PSUM") as pp:
                for g in (range(NG) if do_shared else []):
                    for it in range(SSH // 128):
                        ps_s = pp.tile(
                            [128, 512], f32, name="ps_big2", tag="ps_big", bufs=4
                        )
                        for ct in range(CT):
                            nc.tensor.matmul(
                                out=ps_s[:],
                                lhsT=sw1_sb[:, ct, it * 128 : (it + 1) * 128],
                                rhs=xh_t[g][:, ct, :],
                                start=(ct == 0),
                                stop=(ct == CT - 1),
                            )
                        nc.scalar.activation(
                            out=hs_sb[:, it, g * 512 : (g + 1) * 512],
                            in_=ps_s[:],
                            func=AF.Silu,
                            bias=sb1_sb[:, it : it + 1],
                        )

                def shared_l2(tiles):
                    for tt in tiles:
                        tok = slice(tt * 128, (tt + 1) * 128)
                        orow = sp.tile([128, C], f16, name="sorow", tag="sorow",
                                       bufs=3)
                        for hh in range(2):
                            csl = slice(hh * 512, (hh + 1) * 512)
                            ps2 = pp.tile([128, 512], f32, name="ps_big3",
                                          tag="ps_big", bufs=4)
                            for it in range(SSH // 128):
                                nc.tensor.matmul(
                                    out=ps2[:],
                                    lhsT=hs_sb[:, it, tok],
                                    rhs=sw2_sb[:, it, csl],
                                    start=(it == 0),
                                    stop=(it == SSH // 128 - 1),
                                )
                            nc.vector.tensor_copy(out=orow[:, csl], in_=ps2[:])
                        nc.scalar.dma_start(out=outs[tok, :], in_=orow[:])

                # ---- expert slots (ascending cap order); the tail of the
                # shared expert is emitted after the first gathers so the PE
                # has work while the scatter/gather chain drains ----
                with tc.tile_pool(name="epool", bufs=1) as ep:
                    capmax = max(caps)
                    lbmax = max(lbs)

                    def load_w(s):
                        w1t = ep.tile([128, CT, W], f16, name="w1t", tag="w1t",
                                      bufs=2)
                        nc.sync.dma_start(out=w1t[:], in_=w1s[s, :, :, :])
                        w2t = ep.tile([128, W // 128, C], f16, name="w2t",
                                      tag="w2t", bufs=2)
                        nc.sync.dma_start(out=w2t[:], in_=w2s[s, :, :, :])
                        return w1t, w2t

                    def emit_gathers(s):
                        tiles = []
                        lbcol = int(lbase[s]) // 128
                        for r in range(lbs[s] // 128):
                            xgr = ep.tile([128, C], f16, name="xgr", tag="xgr",
                                          bufs=6)
                            nc.gpsimd.indirect_dma_start(
                                out=xgr[:, :],
                                out_offset=None,
                                in_=xp[:, :],
                                in_offset=bass.IndirectOffsetOnAxis(
                                    ap=toki_all[:, lbcol + r : lbcol + r + 1],
                                    axis=0,
                                ),
                            )
                            tiles.append(xgr)
                        return tiles

                    wpre = {}
                    xgr_pre = {}
                    if do_expert:
                        for s in sorder[:2]:
                            wpre[s] = load_w(s)
                    if do_shared:
                        shared_l2(range(8))
                    if do_expert:
                        for s in sorder[:2]:
                            xgr_pre[s] = emit_gathers(s)
                    if do_shared:
                        shared_l2(range(8, NT))
                    for s in (sorder if do_expert else []):
                        cap, lb = caps[s], lbs[s]
                        ntile = lb // 128
                        w1t, w2t = wpre[s] if s in wpre else load_w(s)
                        xgrs = xgr_pre[s] if s in xgr_pre else emit_gathers(s)
                        xgt = ep.tile([128, CT, lbmax], f16, name="xgt",
                                      tag="xgt", bufs=1)
                        for r in range(ntile):
                            xgr = xgrs[r]
                            for kk in range(CT // 4):
                                ps_t = pp.tile([128, 512], f16, name="ps_tr",
                                               tag="ps_tr", bufs=3)
                                for j in range(4):
                                    ct = kk * 4 + j
                                    nc.tensor.transpose(
                                        out=ps_t[:, j * 128 : (j + 1) * 128],
                                        in_=xgr[:, ct * 128 : (ct + 1) * 128],
                                        identity=ident[:, :128],
                                    )
                                nc.vector.tensor_copy(
                                    out=xgt[
                                        :, kk * 4 : (kk + 1) * 4,
                                        r * 128 : (r + 1) * 128,
                                    ],
                                    in_=ps_t[:],
                                )
                        # L1: h^T = silu(W1^T @ Xg^T + b1)
                        hq = ep.tile([128, W // 128, capmax], f16, name="hq",
                                     tag="hq", bufs=1)
                        for it in range(W // 128):
                            for g0 in range(0, cap, 512):
                                gn = min(512, cap - g0)
                                ps1 = pp.tile([128, 512], f32, name="ps_e1",
                                              tag="ps_big", bufs=4)
                                for ct in range(CT):
                                    nc.tensor.matmul(
                                        out=ps1[:, :gn],
                                        lhsT=w1t[:, ct, it * 128 : (it + 1) * 128],
                                        rhs=xgt[:, ct, g0 : g0 + gn],
                                        start=(ct == 0),
                                        stop=(ct == CT - 1),
                                    )
                                nc.scalar.activation(
                                    out=hq[:, it, g0 : g0 + gn],
                                    in_=ps1[:, :gn],
                                    func=AF.Silu,
                                    bias=b1_sb[:, s, it : it + 1],
                                )
                        # L2: compact output rows (no gate scale, host does it)
                        for t0 in range(0, cap, 128):
                            tn = min(128, cap - t0)
                            orow = ep.tile([128, C], f16, name="eor", tag="eor",
                                           bufs=4)
                            for hh in range(2):
                                csl = slice(hh * 512, (hh + 1) * 512)
                                ps2 = pp.tile([128, 512], f32, name="ps_e2",
                                              tag="ps_big", bufs=4)
                                for it in range(W // 128):
                                    nc.tensor.matmul(
                                        out=ps2[:tn, :],
                                        lhsT=hq[:, it, t0 : t0 + tn],
                                        rhs=w2t[:, it, csl],
                                        start=(it == 0),
                                        stop=(it == W // 128 - 1),
                                    )
                                nc.vector.tensor_copy(
                                    out=orow[:tn, csl], in_=ps2[:tn, :]
                                )
                            nc.scalar.dma_start(
                                out=oce[
                                    int(ocebase[s]) + t0 : int(ocebase[s])
                                    + t0 + tn,
                                    :,
                                ],
                                in_=orow[:tn, :],
                            )

    nc.finalize()
    _BUILD_CACHE[key] = (nc, lbs, ocebase)
    return _BUILD_CACHE[key]


def _pmaj(a):
    """[R, F] -> [128, R//128, F] partition-major slab (large DMA runs)."""
    r, f = a.shape
    return np.ascontiguousarray(a.reshape(r // 128, 128, f).transpose(1, 0, 2))


def _make_in_maps(inputs, p):
    x = np.ascontiguousarray(np.asarray(inputs["x"], np.float32).reshape(N, C))
    xt = x.T                                              # [C, N]
    xth = xt.astype(np.float16)                           # hi part
    xtl = (xt - xth.astype(np.float32)).astype(np.float16)  # lo part
    xhg_np = np.stack(
        [_pmaj(np.ascontiguousarray(xth[:, g * 512 : (g + 1) * 512]))
         for g in range(NG)]
    )
    xlg_np = np.stack(
        [_pmaj(np.ascontiguousarray(xtl[:, g * 512 : (g + 1) * 512]))
         for g in range(NG)]
    )
    xp_np = np.zeros((XPAD, C), np.float16)
    xp_np[:N] = x.astype(np.float16)
    ew1 = np.asarray(inputs["ew1"])
    eb1 = np.asarray(inputs["eb1"])
    ew2 = np.asarray(inputs["ew2"])
    sw1_np = np.asarray(inputs["sw1"])
    sw2_np = np.asarray(inputs["sw2"])
    sb1_np = np.asarray(inputs["sb1"])
    rw1f = np.asarray(inputs["rw1"], np.float32)
    rw1h_np = rw1f.astype(np.float16)
    rw1l_np = (rw1f - rw1h_np.astype(np.float32)).astype(np.float16)
    rw1h_np = _pmaj(rw1h_np)
    rw1l_np = _pmaj(rw1l_np)

    in_maps = []
    for c in range(NCORES):
        isl = slice(c * W, (c + 1) * W)
        w1sp = np.stack(
            [_pmaj(ew1[e][:, isl].astype(np.float16)) for e in range(E)]
        )
        w2sp = np.stack(
            [_pmaj(np.ascontiguousarray(ew2[e][isl, :]).astype(np.float16))
             for e in range(E)]
        )
        in_maps.append(
            {
                "xhg": xhg_np,
                "xlg": xlg_np,
                "xp": xp_np,
                "rw1h": rw1h_np,
                "rw1l": rw1l_np,
                "rb1": np.asarray(inputs["rb1"], np.float32),
                "rw2": np.asarray(inputs["rw2"], np.float32),
                "rb2": np.asarray(inputs["rb2"], np.float32),
                "sw1p": _pmaj(sw1_np[:, isl].astype(np.float16)),
                "sb1s": np.ascontiguousarray(sb1_np[isl].astype(np.float32)),
                "sw2p": _pmaj(
                    np.ascontiguousarray(sw2_np[isl, :]).astype(np.float16)
                ),
                "w1sp": w1sp,
                "b1s": np.ascontiguousarray(eb1[:, isl].astype(np.float32)),
                "w2sp": w2sp,
            }
        )
    return in_maps


def run_spmd(inputs, **kw):
    p = plan(inputs)
    nc, lbs, ocebase = build_nc(p["caps"])
    in_maps = _make_in_maps(inputs, p)
    res = run_bass_kernel_spmd(nc, in_maps, core_ids=list(range(NCORES)), **kw)
    return res, p


def kernel(**inputs) -> np.ndarray:
    p = plan(inputs)
    res, _ = run_spmd(inputs)
    caps = p["caps"]
    gates = p["gates"]                                # [N, E] fp64 softmax
    eb2 = np.asarray(inputs["eb2"], np.float64)       # [E, C]
    sb2 = np.asarray(inputs["sb2"], np.float64)       # [C]

    acc = np.zeros((N, C), np.float64)
    for c in range(NCORES):
        acc += res.results[c]["outs"].astype(np.float64)
    acc += sb2[None, :]

    ocesum = np.zeros((sum(caps), C), np.float64)
    for c in range(NCORES):
        ocesum += res.results[c]["oce"].astype(np.float64)

    # device token lists (identical on every core; use core 0)
    lbs = [_ceil(cap, 128) for cap in caps]
    lbase = np.cumsum([0] + lbs)
    # lstall rows are p-major: row p*NTL+a holds the token at pos a*128+p
    lrows_all = (
        np.asarray(res.results[0]["lstall"]).reshape(128, -1).T.reshape(-1)
    )
    base = 0
    slot_toks, slot_rows = [], []
    sel = np.zeros((N, E), np.float64)
    for s in range(E):
        toks = lrows_all[lbase[s] : lbase[s] + caps[s]].astype(np.int64)
        valid = toks < N
        slot_toks.append(toks[valid])
        rows = ocesum[base : base + caps[s]][valid]
        slot_rows.append(rows)
        sel[toks[valid], s] = 1.0
        base += caps[s]

    # combine weights: softmax(top-k gates / TOPK) over the selected pair
    wexp = np.exp(gates / TOPK) * sel
    wsum = wexp.sum(-1, keepdims=True)
    wsum[wsum == 0] = 1.0
    wn = wexp / wsum
    for s in range(E):
        t = slot_toks[s]
        acc[t] += wn[t, s][:, None] * (slot_rows[s] + eb2[s][None, :])

    return acc.astype(np.float32).reshape(B, T, C)


# revision 34
# speedup vs baseline: 1.3327x; 1.0328x over previous
"""MoE (top-2 of 8 experts, shared expert) Trainium2 Bass kernel, 8-core SPMD.

Strategy (expert parallelism, I-sliced for perfect balance):
 - Slot s on every core processes expert s restricted to the core's I-slice
   [c*512:(c+1)*512].  Every core therefore runs ALL experts on identical
   token counts -> per-core work is equal by construction, and each expert
   weight is loaded exactly once across the machine (fp16, 16MB/core).
 - Router is replicated in exact fp32; the top-2 SELECTION is done on
   LOGITS (exact matmul output), not on softmax gates, so the noisy exp
   activation cannot flip the selection.  The device only produces the
   top-2 mask + compact per-slot token lists; softmax gate values, expert
   biases and the final combine are applied on the host in fp64 from the
   device token lists.
 - Compaction: one batched prefix-sum (3 matmuls for all 8 slots at once)
   computes a global list position for every token's top-1 and top-2
   choice; 32 indirect scatters (128 offsets each, the HW limit) spread
   over 4 independent DRAM lists pipeline on the gpsimd queue; lists are
   merged, written to the lstall output, and reused as gather offsets.
 - Expert FFN in fp16: per-row-tile gathers -> PE transposes (4 per PSUM
   bank) -> L1 silu -> L2 -> compact fp16 output rows.
 - Shared expert is I-sliced 8 ways in fp16 and placed after the
   compaction chains so its matmuls cover the scatter/gather latency.
 - All bulk inputs are shipped partition-major ([128, ct, free] slabs) so
   each DMA descriptor covers 8-16KB; descriptor generation (~11.5ns/desc)
   otherwise caps a queue at ~100GB/s.
 - Host unshards: sums the 8 shared partials and 8 expert partials (the
   I-slices of a token's expert rows add up to the full FFN), applies
   host-softmax gates re-normalized over the device-selected pair.
"""

import os
import sys

sys.path.insert(0, "/opt/trn_rl_repo")

import numpy as np

import concourse.bass as bass
import concourse.mybir as mybir
from concourse import bacc
from concourse.tile import TileContext
from concourse.bass_utils import run_bass_kernel_spmd

f32 = mybir.dt.float32
f16 = mybir.dt.float16
i32 = mybir.dt.int32
AF = mybir.ActivationFunctionType
ALU = mybir.AluOpType

B, T, C, I, E, TOPK = 2, 1024, 1024, 4096, 8, 2
N = B * T                     # 2048 tokens
NCORES = 8
W = I // NCORES               # expert I-slice width per core (512)
SSH = I // NCORES             # shared-expert I-slice width (512)
HR = C // 4                   # router hidden (256)
XPAD = N + 128                # padded token rows; rows >= 2048 are zeros
TRASH_T = float(N)            # trash token id (gathers zeros)
NT = N // 128                 # 16 token tiles
CT = C // 128                 # 8 contraction tiles
NG = N // 512                 # 4 token groups

_BUILD_CACHE = {}


def _ceil(x, g):
    return -(-x // g) * g


def plan(inputs):
    """Host-side routing estimate: exact caps + gates for the combine.

    The fp32 numpy router matches the device fp32 router selection: the
    smallest top2-vs-top3 logit gap for this input is ~1.3e-4, vastly
    above both computations' noise.
    """
    x = np.asarray(inputs["x"], np.float32).reshape(N, C)
    h = np.maximum(x @ np.asarray(inputs["rw1"]) + np.asarray(inputs["rb1"]), 0)
    logits = (h @ np.asarray(inputs["rw2"]) + np.asarray(inputs["rb2"])).astype(
        np.float64
    )
    g = np.exp(logits - logits.max(-1, keepdims=True))
    g /= g.sum(-1, keepdims=True)
    top2 = np.argsort(-logits, axis=-1)[:, :TOPK]
    counts = np.bincount(top2.ravel(), minlength=E)
    caps = [max(128, _ceil(int(c) + 4, 64)) for c in counts]
    return {"caps": caps, "counts": counts, "gates": g}


def build_nc(caps):
    key = (tuple(caps), os.environ.get("MOE_STOP", ""))
    if key in _BUILD_CACHE:
        return _BUILD_CACHE[key]

    lbs = [_ceil(cap, 128) for cap in caps]          # list/gather rows per slot
    ocebase = np.cumsum([0] + caps)                  # oce row offsets
    OCER = int(ocebase[-1])
    lbase = np.cumsum([0] + lbs)                     # list section offsets
    LBTOT = int(lbase[-1])
    # expert slot processing order: small caps first so weight DMA keeps up
    sorder = sorted(range(E), key=lambda s: caps[s])

    stop = os.environ.get("MOE_STOP", "")
    do_compact = stop != "router"
    do_shared = do_compact and stop != "compact"
    do_expert = do_shared and stop != "shared"

    nc = bacc.Bacc("TRN2", target_bir_lowering=False)

    # ---------------- I/O (bulk tensors partition-major) ----------------
    xhg = nc.dram_tensor("xhg", [NG, 128, CT, 512], f16, kind="ExternalInput")
    xlg = nc.dram_tensor("xlg", [NG, 128, CT, 512], f16, kind="ExternalInput")
    xp = nc.dram_tensor("xp", [XPAD, C], f16, kind="ExternalInput")
    rw1h = nc.dram_tensor("rw1h", [128, CT, HR], f16, kind="ExternalInput")
    rw1l = nc.dram_tensor("rw1l", [128, CT, HR], f16, kind="ExternalInput")
    rb1 = nc.dram_tensor("rb1", [HR], f32, kind="ExternalInput")
    rw2 = nc.dram_tensor("rw2", [HR, E], f32, kind="ExternalInput")
    rb2 = nc.dram_tensor("rb2", [E], f32, kind="ExternalInput")
    sw1 = nc.dram_tensor("sw1p", [128, CT, SSH], f16, kind="ExternalInput")
    sb1 = nc.dram_tensor("sb1s", [SSH], f32, kind="ExternalInput")
    sw2 = nc.dram_tensor("sw2p", [128, SSH // 128, C], f16, kind="ExternalInput")
    w1s = nc.dram_tensor("w1sp", [E, 128, CT, W], f16, kind="ExternalInput")
    b1s = nc.dram_tensor("b1s", [E, W], f32, kind="ExternalInput")
    w2s = nc.dram_tensor("w2sp", [E, 128, W // 128, C], f16, kind="ExternalInput")

    outs = nc.dram_tensor("outs", [N, C], f16, kind="ExternalOutput")
    oce = nc.dram_tensor("oce", [OCER, C], f16, kind="ExternalOutput")
    lstall = nc.dram_tensor("lstall", [LBTOT, 1], f32, kind="ExternalOutput")

    # ---------------- compile-time constants ----------------
    ut128_np = (np.arange(128)[:, None] < np.arange(128)[None, :]).astype(np.float32)
    iota_np = (np.arange(NT)[None, :] * 128 + np.arange(128)[:, None]).astype(
        np.float32
    )
    fill_np = np.full((128, 40), TRASH_T, np.float32)
    ident_np = np.zeros((128, 256), dtype=np.float16)
    ident_np[:, :128] = np.eye(128, dtype=np.float16)
    # (c,s)-flattened strict-upper tile-prefix mask: contributes t_(c',s) to
    # (c,s) iff same slot and c' < c
    cs_c = np.arange(128) // E
    cs_s = np.arange(128) % E
    utcs_np = (
        (cs_s[:, None] == cs_s[None, :]) & (cs_c[:, None] < cs_c[None, :])
    ).astype(np.float32)
    lbrow_np = np.array([lbase[s] for s in cs_s], np.float32)[None, :]
    ut128_d = nc.inline_tensor(ut128_np, "ut128c")
    utcs_d = nc.inline_tensor(utcs_np, "utcsc")
    lbrow_d = nc.inline_tensor(lbrow_np, "lbrowc")
    iota_d = nc.inline_tensor(iota_np, "iotac")
    fill_d = nc.inline_tensor(fill_np, "fillc")
    ident_d = nc.inline_tensor(ident_np, "identc")
    ones128_d = nc.inline_tensor(np.ones((128, 1), np.float32), "ones128c")
    onesrow_d = nc.inline_tensor(np.ones((1, 128), np.float32), "onesrowc")

    with TileContext(nc) as tc:
        with (
            tc.tile_pool(name="cpool", bufs=1) as cp,
            tc.tile_pool(name="mpool", bufs=1) as mp,
            tc.tile_pool(name="spool", bufs=1) as sp,
        ):
            # ---- constants into SBUF (scalar queue; sync stays clear) ----
            rw1h_sb = cp.tile([128, CT, HR], f16, name="rw1h_sb")
            nc.scalar.dma_start(out=rw1h_sb[:], in_=rw1h[:, :, :])
            rw1l_sb = cp.tile([128, CT, HR], f16, name="rw1l_sb")
            nc.scalar.dma_start(out=rw1l_sb[:], in_=rw1l[:, :, :])
            ut128 = cp.tile([128, 128], f32, name="ut128")
            nc.scalar.dma_start(out=ut128[:], in_=ut128_d[:, :])
            utcs = cp.tile([128, 128], f32, name="utcs")
            nc.scalar.dma_start(out=utcs[:], in_=utcs_d[:, :])
            lbrow = cp.tile([1, 128], f32, name="lbrow")
            nc.scalar.dma_start(out=lbrow[:], in_=lbrow_d[:, :])
            iota = cp.tile([128, NT], f32, name="iota")
            nc.scalar.dma_start(out=iota[:], in_=iota_d[:, :])
            fill = cp.tile([128, 40], f32, name="fill")
            nc.scalar.dma_start(out=fill[:], in_=fill_d[:, :])
            ident = cp.tile([128, 256], f16, name="ident")
            nc.scalar.dma_start(out=ident[:], in_=ident_d[:, :])
            ones128 = cp.tile([128, 1], f32, name="ones128")
            nc.scalar.dma_start(out=ones128[:], in_=ones128_d[:, :])
            onesrow = cp.tile([1, 128], f32, name="onesrow")
            nc.scalar.dma_start(out=onesrow[:], in_=onesrow_d[:, :])
            rb1_sb = cp.tile([128, HR // 128], f32, name="rb1_sb")
            nc.scalar.dma_start(
                out=rb1_sb[:], in_=rb1.rearrange("(a p) -> p a", p=128)
            )
            rw2_sb = cp.tile([128, HR // 128, E], f32, name="rw2_sb")
            nc.scalar.dma_start(
                out=rw2_sb[:], in_=rw2.rearrange("(a p) e -> p a e", p=128)
            )
            rb2_row = cp.tile([1, E], f32, name="rb2_row")
            nc.scalar.dma_start(out=rb2_row[:], in_=rb2[None, :])
            sb1_sb = cp.tile([128, SSH // 128], f32, name="sb1_sb")
            nc.scalar.dma_start(
                out=sb1_sb[:], in_=sb1.rearrange("(a p) -> p a", p=128)
            )
            b1_sb = cp.tile([128, E, W // 128], f32, name="b1_sb")
            nc.scalar.dma_start(
                out=b1_sb[:], in_=b1s.rearrange("s (a p) -> p s a", p=128)
            )

            wall = cp.tile([128, NT, E], f32, name="wall")
            wall1 = cp.tile([128, NT, E], f32, name="wall1")
            wall2 = cp.tile([128, NT, E], f32, name="wall2")
            nc.vector.memset(wall[:], 0.0)
            nc.vector.memset(wall1[:], 0.0)
            toki_all = cp.tile([128, LBTOT // 128], i32, name="toki_all")

            # shared-expert tiles allocated up-front (addresses distinct from
            # the router pool so their DMAs are not blocked by address reuse)
            hs_sb = sp.tile([128, SSH // 128, N], f16, name="hs_sb")
            sw1_sb = sp.tile([128, CT, SSH], f16, name="sw1_sb")
            sw2_sb = sp.tile([128, SSH // 128, C], f16, name="sw2_sb")
            xh_t = [
                sp.tile([128, CT, 512], f16, name=f"xh{g}", tag=f"xh{g}")
                for g in range(NG)
            ]

            # ---- compaction scaffolding (PSUM + DRAM pools coexist with
            # the router pools; scatters start as soon as the first 8
            # epilogue tiles are done) ----
            NCH = 8
            cpp_ctx = tc.tile_pool(name="cpp", bufs=1, space="PSUM")
            cpp = cpp_ctx.__enter__()
            dp_ctx = tc.tile_pool(name="dpool", bufs=1, space="DRAM")
            dp = dp_ctx.__enter__()
            lsts = []
            if do_compact:
                for k in range(NCH):
                    lk = dp.tile([LBTOT, 1], f32, name=f"lst{k}", tag=f"lst{k}")
                    lsts.append(lk)
                    nc.scalar.dma_start(
                        out=lk[:, :].rearrange("(p a) x -> p (a x)", p=128),
                        in_=fill[:, : LBTOT // 128],
                    )
            NTL = LBTOT // 128

            def compact_phase(cols, ph):
                """Batched rank computation on the CURRENT wall (columns not
                yet written are zero), then scatter the given columns.  The
                rank of a token in column c only involves columns <= c, so
                phase A (cols 0-7) is exact before tiles 8-15 exist."""
                nc.vector.tensor_sub(wall2[:, :, :], wall[:, :, :],
                                     wall1[:, :, :])
                ps_pre = cpp.tile([128, NT, E], f32, name=f"ps_pre{ph}",
                                  tag="ps_pre", bufs=1)
                nc.tensor.matmul(
                    out=ps_pre[:, :, :], lhsT=ut128[:], rhs=wall[:, :, :],
                    start=True, stop=False,
                )
                ps_tot = cpp.tile([128, 1], f32, name=f"ps_tot{ph}",
                                  tag="ps_tot", bufs=1)
                nc.tensor.matmul(
                    out=ps_tot[:], lhsT=wall[:, :, :], rhs=ones128[:],
                    start=True, stop=True,
                )
                tot_sb = mp.tile([128, 1], f32, name=f"tot{ph}", tag=f"tot{ph}")
                nc.vector.tensor_copy(out=tot_sb[:], in_=ps_tot[:])
                ps_pt = cpp.tile([1, 128], f32, name=f"ps_pt{ph}", tag="ps_pt",
                                 bufs=1)
                nc.tensor.matmul(
                    out=ps_pt[:], lhsT=tot_sb[:], rhs=utcs[:],
                    start=True, stop=True,
                )
                ptot_row = mp.tile([1, 128], f32, name=f"ptr{ph}", tag=f"ptr{ph}")
                nc.vector.tensor_add(ptot_row[:], ps_pt[:], lbrow[:])
                nc.tensor.matmul(
                    out=ps_pre[:, :, :], lhsT=onesrow[:], rhs=ptot_row[:],
                    start=False, stop=True,
                )
                tmp1 = mp.tile([128, NT, E], f32, name=f"tmp1{ph}",
                               tag=f"tmp1{ph}")
                nc.vector.tensor_mul(tmp1[:], ps_pre[:, :, :], wall1[:, :, :])
                pos1 = mp.tile([128, NT, 1], f32, name=f"pos1{ph}",
                               tag=f"pos1{ph}")
                nc.vector.tensor_reduce(
                    out=pos1[:], in_=tmp1[:], axis=mybir.AxisListType.X,
                    op=ALU.add,
                )
                tmp2 = mp.tile([128, NT, E], f32, name=f"tmp2{ph}",
                               tag=f"tmp2{ph}")
                nc.vector.tensor_mul(tmp2[:], ps_pre[:, :, :], wall2[:, :, :])
                pos2 = mp.tile([128, NT, 1], f32, name=f"pos2{ph}",
                               tag=f"pos2{ph}")
                nc.vector.tensor_reduce(
                    out=pos2[:], in_=tmp2[:], axis=mybir.AxisListType.X,
                    op=ALU.add,
                )
                cnt2 = mp.tile([128, NT, 1], f32, name=f"cnt2{ph}",
                               tag=f"cnt2{ph}")
                nc.vector.tensor_reduce(
                    out=cnt2[:], in_=wall2[:, :, :], axis=mybir.AxisListType.X,
                    op=ALU.add,
                )
                guard = mp.tile([128, NT, 1], f32, name=f"guard{ph}",
                                tag=f"guard{ph}")
                nc.vector.tensor_scalar(
                    guard[:], cnt2[:], -1.0e9, 1.0e9, op0=ALU.mult,
                    op1=ALU.add,
                )
                nc.vector.tensor_add(pos2[:], pos2[:], guard[:])
                posis = []
                for nmo, possrc in ((f"p1{ph}", pos1), (f"p2{ph}", pos2)):
                    pint = mp.tile([128, NT], i32, name=f"{nmo}i",
                                   tag=f"{nmo}i")
                    nc.vector.tensor_copy(out=pint[:], in_=possrc[:, :, 0])
                    hi = mp.tile([128, NT], i32, name=f"{nmo}h", tag=f"{nmo}h")
                    nc.vector.tensor_scalar(
                        hi[:], pint[:], 7, None, op0=ALU.logical_shift_right
                    )
                    lo = mp.tile([128, NT], i32, name=f"{nmo}l", tag=f"{nmo}l")
                    nc.vector.tensor_scalar(
                        lo[:], pint[:], 127, None, op0=ALU.bitwise_and
                    )
                    nc.vector.tensor_scalar(
                        lo[:], lo[:], NTL, None, op0=ALU.mult
                    )
                    posi = mp.tile([128, NT], i32, name=nmo, tag=nmo)
                    nc.vector.tensor_add(posi[:], lo[:], hi[:])
                    posis.append(posi)
                for c in cols:
                    for ch, posi in (
                        (c % (NCH // 2), posis[0]),
                        (NCH // 2 + c % (NCH // 2), posis[1]),
                    ):
                        nc.gpsimd.indirect_dma_start(
                            out=lsts[ch][:, :],
                            out_offset=bass.IndirectOffsetOnAxis(
                                ap=posi[:, c : c + 1], axis=0
                            ),
                            in_=iota[:, c : c + 1],
                            in_offset=None,
                            bounds_check=LBTOT - 1,
                            oob_is_err=False,
                        )

            # ---- phase R: router (fp16 hi/lo pair, fp32-class exact) ----
            with (
                tc.tile_pool(name="rpool", bufs=1) as rp,
                tc.tile_pool(name="rpp", bufs=1, space="PSUM") as pp,
            ):
                hr_sb = rp.tile([128, HR // 128, N], f32, name="hr_sb")
                xl_t = []
                for g in range(NG):
                    nc.sync.dma_start(out=xh_t[g][:], in_=xhg[g, :, :, :])
                    xl_g = rp.tile([128, CT, 512], f16, name=f"xl{g}", tag=f"xl{g}")
                    nc.sync.dma_start(out=xl_g[:], in_=xlg[g, :, :, :])
                    xl_t.append(xl_g)
                # shared-expert weight loads queue right behind the router's
                nc.sync.dma_start(out=sw1_sb[:], in_=sw1[:, :, :])
                nc.sync.dma_start(out=sw2_sb[:], in_=sw2[:, :, :])

                # x = hi + lo (fp16 pair); x@w = hi@w_hi + lo@w_hi + hi@w_lo
                # to fp32-class accuracy at 3 fp16 passes (vs 4 for fp32)
                for g in range(NG):
                    for ht in range(HR // 128):
                        ps_h = pp.tile(
                            [128, 512], f32, name="ps_big", tag="ps_big", bufs=3
                        )
                        hsl = slice(ht * 128, (ht + 1) * 128)
                        for ct in range(CT):
                            nc.tensor.matmul(
                                out=ps_h[:],
                                lhsT=rw1h_sb[:, ct, hsl],
                                rhs=xh_t[g][:, ct, :],
                                start=(ct == 0),
                                stop=False,
                            )
                            nc.tensor.matmul(
                                out=ps_h[:],
                                lhsT=rw1h_sb[:, ct, hsl],
                                rhs=xl_t[g][:, ct, :],
                                start=False,
                                stop=False,
                            )
                            nc.tensor.matmul(
                                out=ps_h[:],
                                lhsT=rw1l_sb[:, ct, hsl],
                                rhs=xh_t[g][:, ct, :],
                                start=False,
                                stop=(ct == CT - 1),
                            )
                        nc.scalar.activation(
                            out=hr_sb[:, ht, g * 512 : (g + 1) * 512],
                            in_=ps_h[:],
                            func=AF.Relu,
                            bias=rb1_sb[:, ht : ht + 1],
                        )

                # router L2 + top-2-on-logits epilogue (4 psum tiles deep)
                for tt in range(NT):
                    tok = slice(tt * 128, (tt + 1) * 128)
                    ps_l = pp.tile([128, E], f32, name="ps_l", tag="ps_l", bufs=2)
                    for ht in range(HR // 128):
                        nc.tensor.matmul(
                            out=ps_l[:],
                            lhsT=hr_sb[:, ht, tok],
                            rhs=rw2_sb[:, ht, :],
                            start=(ht == 0),
                            stop=False,
                        )
                    nc.tensor.matmul(
                        out=ps_l[:],
                        lhsT=onesrow[:],
                        rhs=rb2_row[:],
                        start=False,
                        stop=True,
                    )
                    lg = mp.tile([128, E], f32, name="lg", tag="lg", bufs=4)
                    nc.scalar.copy(out=lg[:], in_=ps_l[:])
                    mxl = mp.tile([128, 8], f32, name="mxl", tag="mxl", bufs=4)
                    nc.vector.max(out=mxl[:], in_=lg[:])
                    nc.vector.tensor_scalar(
                        wall[:, tt, :], lg[:], mxl[:, 1:2], None, op0=ALU.is_ge
                    )
                    nc.vector.tensor_scalar(
                        wall1[:, tt, :], lg[:], mxl[:, 0:1], None, op0=ALU.is_ge
                    )
                    if do_shared:
                        g, it = tt // 4, tt % 4
                        ps_s = pp.tile(
                            [128, 512], f32, name="ps_sl1", tag="ps_big", bufs=3
                        )
                        for ct in range(CT):
                            nc.tensor.matmul(
                                out=ps_s[:],
                                lhsT=sw1_sb[:, ct, it * 128 : (it + 1) * 128],
                                rhs=xh_t[g][:, ct, :],
                                start=(ct == 0),
                                stop=(ct == CT - 1),
                            )
                        nc.scalar.activation(
                            out=hs_sb[:, it, g * 512 : (g + 1) * 512],
                            in_=ps_s[:],
                            func=AF.Silu,
                            bias=sb1_sb[:, it : it + 1],
                        )
                    if do_compact and tt == 7:
                        compact_phase(range(0, 8), "A")
                    if do_compact and tt == NT - 1:
                        compact_phase(range(8, NT), "B")

            # ---- list readback + merge -> gather offsets ----
            if do_compact:
                iws = []
                for k in range(NCH):
                    iwk = mp.tile([128, LBTOT // 128], f32, name=f"iw{k}",
                                  tag=f"iw{k}")
                    iws.append(iwk)
                    nc.gpsimd.dma_start(
                        out=iwk[:, :],
                        in_=lsts[k][:, :].rearrange("(p a) x -> p (a x)", p=128),
                    )
                iwm = mp.tile([128, LBTOT // 128], f32, name="iwm")
                nc.vector.tensor_add(iwm[:], iws[0][:], iws[1][:])
                for k in range(2, NCH):
                    nc.vector.tensor_add(iwm[:], iwm[:], iws[k][:])
                nc.vector.tensor_scalar_add(iwm[:], iwm[:], -(NCH - 1.0) * N)
                nc.vector.tensor_copy(out=toki_all[:, :], in_=iwm[:])
                nc.sync.dma_start(
                    out=lstall.rearrange("(p a) x -> p (a x)", p=128),
                    in_=iwm[:, :],
                )
            dp_ctx.__exit__(None, None, None)
            cpp_ctx.__exit__(None, None, None)

            # ---- shared expert (fp16, I-slice): fills the scatter/gather
            # latency; then expert slots ----
            with tc.tile_pool(name="spp", bufs=1, space="PSUM") as pp:
                def shared_l2(tiles):
                    for tt in tiles:
                        tok = slice(tt * 128, (tt + 1) * 128)
                        orow = sp.tile([128, C], f16, name="sorow", tag="sorow",
                                       bufs=3)
                        for hh in range(2):
                            csl = slice(hh * 512, (hh + 1) * 512)
                            ps2 = pp.tile([128, 512], f32, name="ps_big3",
                                          tag="ps_big", bufs=4)
                            for it in range(SSH // 128):
                                nc.tensor.matmul(
                                    out=ps2[:],
                                    lhsT=hs_sb[:, it, tok],
                                    rhs=sw2_sb[:, it, csl],
                                    start=(it == 0),
                                    stop=(it == SSH // 128 - 1),
                                )
                            nc.vector.tensor_copy(out=orow[:, csl], in_=ps2[:])
                        nc.scalar.dma_start(out=outs[tok, :], in_=orow[:])

                # ---- expert slots (ascending cap order); the tail of the
                # shared expert is emitted after the first gathers so the PE
                # has work while the scatter/gather chain drains ----
                with tc.tile_pool(name="epool", bufs=1) as ep:
                    capmax = max(caps)
                    lbmax = max(lbs)

                    def load_w(s):
                        w1t = ep.tile([128, CT, W], f16, name="w1t", tag="w1t",
                                      bufs=2)
                        nc.sync.dma_start(out=w1t[:], in_=w1s[s, :, :, :])
                        w2t = ep.tile([128, W // 128, C], f16, name="w2t",
                                      tag="w2t", bufs=2)
                        nc.sync.dma_start(out=w2t[:], in_=w2s[s, :, :, :])
                        return w1t, w2t

                    def emit_gathers(s):
                        tiles = []
                        lbcol = int(lbase[s]) // 128
                        for r in range(lbs[s] // 128):
                            xgr = ep.tile([128, C], f16, name="xgr", tag="xgr",
                                          bufs=6)
                            nc.gpsimd.indirect_dma_start(
                                out=xgr[:, :],
                                out_offset=None,
                                in_=xp[:, :],
                                in_offset=bass.IndirectOffsetOnAxis(
                                    ap=toki_all[:, lbcol + r : lbcol + r + 1],
                                    axis=0,
                                ),
                            )
                            tiles.append(xgr)
                        return tiles

                    wpre = {}
                    xgr_pre = {}
                    if do_expert:
                        for s in sorder[:2]:
                            wpre[s] = load_w(s)
                    if do_shared:
                        shared_l2(range(8))
                    if do_expert:
                        for s in sorder[:2]:
                            xgr_pre[s] = emit_gathers(s)
                    if do_shared:
                        shared_l2(range(8, NT))
                    for s in (sorder if do_expert else []):
                        cap, lb = caps[s], lbs[s]
                        ntile = lb // 128
                        w1t, w2t = wpre[s] if s in wpre else load_w(s)
                        xgrs = xgr_pre[s] if s in xgr_pre else emit_gathers(s)
                        xgt = ep.tile([128, CT, lbmax], f16, name="xgt",
                                      tag="xgt", bufs=1)
                        for r in range(ntile):
                            xgr = xgrs[r]
                            for kk in range(CT // 4):
                                ps_t = pp.tile([128, 512], f16, name="ps_tr",
                                               tag="ps_tr", bufs=3)
                                for j in range(4):
                                    ct = kk * 4 + j
                                    nc.tensor.transpose(
                                        out=ps_t[:, j * 128 : (j + 1) * 128],
                                        in_=xgr[:, ct * 128 : (ct + 1) * 128],
                                        identity=ident[:, :128],
                                    )
                                nc.vector.tensor_copy(
                                    out=xgt[
                                        :, kk * 4 : (kk + 1) * 4,
                                        r * 128 : (r + 1) * 128,
                                    ],
                                    in_=ps_t[:],
                                )
                        # L1: h^T = silu(W1^T @ Xg^T + b1)
                        hq = ep.tile([128, W // 128, capmax], f16, name="hq",
                                     tag="hq", bufs=1)
                        for it in range(W // 128):
                            for g0 in range(0, cap, 512):
                                gn = min(512, cap - g0)
                                ps1 = pp.tile([128, 512], f32, name="ps_e1",
                                              tag="ps_big", bufs=4)
                                for ct in range(CT):
                                    nc.tensor.matmul(
                                        out=ps1[:, :gn],
                                        lhsT=w1t[:, ct, it * 128 : (it + 1) * 128],
                                        rhs=xgt[:, ct, g0 : g0 + gn],
                                        start=(ct == 0),
                                        stop=(ct == CT - 1),
                                    )
                                nc.scalar.activation(
                                    out=hq[:, it, g0 : g0 + gn],
                                    in_=ps1[:, :gn],
                                    func=AF.Silu,
                                    bias=b1_sb[:, s, it : it + 1],
                                )
                        # L2: compact output rows (no gate scale, host does it)
                        for t0 in range(0, cap, 128):
                            tn = min(128, cap - t0)
                            orow = ep.tile([128, C], f16, name="eor", tag="eor",
                                           bufs=4)
                            for hh in range(2):
                                csl = slice(hh * 512, (hh + 1) * 512)
                                ps2 = pp.tile([128, 512], f32, name="ps_e2",
                                              tag="ps_big", bufs=4)
                                for it in range(W // 128):
                                    nc.tensor.matmul(
                                        out=ps2[:tn, :],
                                        lhsT=hq[:, it, t0 : t0 + tn],
                                        rhs=w2t[:, it, csl],
                                        start=(it == 0),
                                        stop=(it == W // 128 - 1),
                                    )
                                nc.vector.tensor_copy(
                                    out=orow[:tn, csl], in_=ps2[:tn, :]
                                )
                            nc.scalar.dma_start(
                                out=oce[
                                    int(ocebase[s]) + t0 : int(ocebase[s])
                                    + t0 + tn,
                                    :,
                                ],
                                in_=orow[:tn, :],
                            )

    nc.finalize()
    _BUILD_CACHE[key] = (nc, lbs, ocebase)
    return _BUILD_CACHE[key]


def _pmaj(a):
    """[R, F] -> [128, R//128, F] partition-major slab (large DMA runs)."""
    r, f = a.shape
    return np.ascontiguousarray(a.reshape(r // 128, 128, f).transpose(1, 0, 2))


def _make_in_maps(inputs, p):
    x = np.ascontiguousarray(np.asarray(inputs["x"], np.float32).reshape(N, C))
    xt = x.T                                              # [C, N]
    xth = xt.astype(np.float16)                           # hi part
    xtl = (xt - xth.astype(np.float32)).astype(np.float16)  # lo part
    xhg_np = np.stack(
        [_pmaj(np.ascontiguousarray(xth[:, g * 512 : (g + 1) * 512]))
         for g in range(NG)]
    )
    xlg_np = np.stack(
        [_pmaj(np.ascontiguousarray(xtl[:, g * 512 : (g + 1) * 512]))
         for g in range(NG)]
    )
    xp_np = np.zeros((XPAD, C), np.float16)
    xp_np[:N] = x.astype(np.float16)
    ew1 = np.asarray(inputs["ew1"])
    eb1 = np.asarray(inputs["eb1"])
    ew2 = np.asarray(inputs["ew2"])
    sw1_np = np.asarray(inputs["sw1"])
    sw2_np = np.asarray(inputs["sw2"])
    sb1_np = np.asarray(inputs["sb1"])
    rw1f = np.asarray(inputs["rw1"], np.float32)
    rw1h_np = rw1f.astype(np.float16)
    rw1l_np = (rw1f - rw1h_np.astype(np.float32)).astype(np.float16)
    rw1h_np = _pmaj(rw1h_np)
    rw1l_np = _pmaj(rw1l_np)

    in_maps = []
    for c in range(NCORES):
        isl = slice(c * W, (c + 1) * W)
        w1sp = np.stack(
            [_pmaj(ew1[e][:, isl].astype(np.float16)) for e in range(E)]
        )
        w2sp = np.stack(
            [_pmaj(np.ascontiguousarray(ew2[e][isl, :]).astype(np.float16))
             for e in range(E)]
        )
        in_maps.append(
            {
                "xhg": xhg_np,
                "xlg": xlg_np,
                "xp": xp_np,
                "rw1h": rw1h_np,
                "rw1l": rw1l_np,
                "rb1": np.asarray(inputs["rb1"], np.float32),
                "rw2": np.asarray(inputs["rw2"], np.float32),
                "rb2": np.asarray(inputs["rb2"], np.float32),
                "sw1p": _pmaj(sw1_np[:, isl].astype(np.float16)),
                "sb1s": np.ascontiguousarray(sb1_np[isl].astype(np.float32)),
                "sw2p": _pmaj(
                    np.ascontiguousarray(sw2_np[isl, :]).astype(np.float16)
                ),
                "w1sp": w1sp,
                "b1s": np.ascontiguousarray(eb1[:, isl].astype(np.float32)),
                "w2sp": w2sp,
            }
        )
    return in_maps


def run_spmd(inputs, **kw):
    p = plan(inputs)
    nc, lbs, ocebase = build_nc(p["caps"])
    in_maps = _make_in_maps(inputs, p)
    res = run_bass_kernel_spmd(nc, in_maps, core_ids=list(range(NCORES)), **kw)
    return res, p


def kernel(**inputs) -> np.ndarray:
    p = plan(inputs)
    res, _ = run_spmd(inputs)
    caps = p["caps"]
    gates = p["gates"]                                # [N, E] fp64 softmax
    eb2 = np.asarray(inputs["eb2"], np.float64)       # [E, C]
    sb2 = np.asarray(inputs["sb2"], np.float64)       # [C]

    acc = np.zeros((N, C), np.float64)
    for c in range(NCORES):
        acc += res.results[c]["outs"].astype(np.float64)
    acc += sb2[None, :]

    ocesum = np.zeros((sum(caps), C), np.float64)
    for c in range(NCORES):
        ocesum += res.results[c]["oce"].astype(np.float64)

    # device token lists (identical on every core; use core 0)
    lbs = [_ceil(cap, 128) for cap in caps]
    lbase = np.cumsum([0] + lbs)
    # lstall rows are p-major: row p*NTL+a holds the token at pos a*128+p
    lrows_all = (
        np.asarray(res.results[0]["lstall"]).reshape(128, -1).T.reshape(-1)
    )
    base = 0
    slot_toks, slot_rows = [], []
    sel = np.zeros((N, E), np.float64)
    for s in range(E):
        toks = lrows_all[lbase[s] : lbase[s] + caps[s]].astype(np.int64)
        valid = toks < N
        slot_toks.append(toks[valid])
        rows = ocesum[base : base + caps[s]][valid]
        slot_rows.append(rows)
        sel[toks[valid], s] = 1.0
        base += caps[s]

    # combine weights: softmax(top-k gates / TOPK) over the selected pair
    wexp = np.exp(gates / TOPK) * sel
    wsum = wexp.sum(-1, keepdims=True)
    wsum[wsum == 0] = 1.0
    wn = wexp / wsum
    for s in range(E):
        t = slot_toks[s]
        acc[t] += wn[t, s][:, None] * (slot_rows[s] + eb2[s][None, :])

    return acc.astype(np.float32).reshape(B, T, C)
